# revision 11
# baseline (speedup 1.0000x reference)
"""Trainium2 Bass kernel for nn_DualAxisAggAttn (dual-axis aggregation attention).

Reference semantics per batch image x[C=256, H=64, W=64], twice (W axis then H axis):
  qkv = conv1x1(x) -> {q:[1], k:[C], v:[C]};  s = softmax_axis(q)
  ctx[c,a] = sum_r k*s;  out = x + sigmoid(v) * ctx_bcast;  y = conv1x1(out)

Distribution: data-parallel over batch (16 images -> 2 per NeuronCore x 8 cores).

v3 structure (vs the 129us baseline):
  - STAGE FOLD: stage H is linear in y_W before each nonlinearity, so the
    stage-W fusion conv folds into stage-H weights host-side:
      qkvH' = qkvH @ WfW,  Wff = WfH @ WfW,  biases folded likewise.
    Stage H consumes xeffW = x + gW*ctxW directly; the stage-W fusion
    matmul (1/3 of all PE work) and its PSUM evictions vanish.
  - key-path linearity: ctx = Wk @ (sum_r x*E) / S (key conv after reduction).
  - all matmuls bf16 (fp8 DoubleRow measured at the same col/cycle rate as
    bf16 on HW, so it only costs precision).
  - reductions: image-level halving trees with every level writing a fresh
    scratch region (in-place/strided variants measured 2x; fresh-dest
    [64,n]-shaped levels hit the 4x DVE mode).
  - elementwise load split across engines: u-mult and xeff-combine chunks
    alternate DVE <-> GpSimd(Pool) queue; g2 (scalar_tensor_tensor, 1x,
    DVE-only op) stays on DVE; psum evictions all on ACT.
  - sigmoid via tanh ((1+tanh(v/2))/2): exp+tanh share one ACT table set;
    0.5 folds into ctx scale, +1 into the g2 scalar_tensor_tensor.
"""

import numpy as np
import ml_dtypes
from contextlib import ExitStack

import concourse.bass as bass
import concourse.bacc as bacc
import concourse.tile as tile
import concourse.mybir as mybir
from concourse.bass_utils import run_bass_kernel_spmd

F32 = mybir.dt.float32
BF16 = mybir.dt.bfloat16
AF = mybir.ActivationFunctionType
ALU = mybir.AluOpType
AX = mybir.AxisListType
NPBF = ml_dtypes.bfloat16

B, C, H, W = 16, 256, 64, 64
HW = H * W
NCORES = 8
BPC = B // NCORES
KT = 2
CH = 512
NCH = HW // CH
GRP = CH // 64

# Pool-queue offload masks (chunk j goes to GpSimd when bit set)
GP_U_MASK = 0xAA      # u-mult chunks on Pool
GP_XE_MASK = 0xAA     # xeff chunks on Pool

_BUILD_CACHE = {}
LAST_RESULTS = None


def _build(flags):
    bvW0, bkW0, bqW0, bvH0, bkH0, bqH0, by0 = flags
    nc = bacc.Bacc(trn_type="TRN2", target_bir_lowering=False, debug=False)

    xbf_d = nc.dram_tensor("xbf", [BPC, KT, 128, HW], BF16, kind="ExternalInput").ap()
    statW_d = nc.dram_tensor("statW", [128, KT, 3, 128], BF16, kind="ExternalInput").ap()
    statH_d = nc.dram_tensor("statH", [128, KT, 3, 128], BF16, kind="ExternalInput").ap()
    wkW_d = nc.dram_tensor("wkW", [128, KT, 2, 128], BF16, kind="ExternalInput").ap()
    wkH_d = nc.dram_tensor("wkH", [128, KT, 2, 128], BF16, kind="ExternalInput").ap()
    fusA_d = nc.dram_tensor("fusA", [128, KT, 2, 128], BF16, kind="ExternalInput").ap()
    fusB_d = nc.dram_tensor("fusB", [128, KT, 2, 128], BF16, kind="ExternalInput").ap()
    bias_d = nc.dram_tensor("biases", [7, 2, 128], F32, kind="ExternalInput").ap()
    y_d = nc.dram_tensor("y", [BPC, C, HW], BF16, kind="ExternalOutput").ap()

    with tile.TileContext(nc) as tc, ExitStack() as ctx:
        wp = ctx.enter_context(tc.tile_pool(name="weights", bufs=1))
        pxb = ctx.enter_context(tc.tile_pool(name="xb", bufs=2))
        pxe = ctx.enter_context(tc.tile_pool(name="xe", bufs=2))
        pT = ctx.enter_context(tc.tile_pool(name="T", bufs=2))
        pg = ctx.enter_context(tc.tile_pool(name="gate", bufs=2))
        pacc = ctx.enter_context(tc.tile_pool(name="acc", bufs=2))
        psm = ctx.enter_context(tc.tile_pool(name="small", bufs=8))
        pch = ctx.enter_context(tc.tile_pool(name="chunk", bufs=3))
        phv = ctx.enter_context(tc.tile_pool(name="hv", bufs=1))
        pyv = ctx.enter_context(tc.tile_pool(name="yev", bufs=2))
        pq = ctx.enter_context(tc.tile_pool(name="psq", bufs=2, space="PSUM"))
        pvf = ctx.enter_context(tc.tile_pool(name="psvf", bufs=3, space="PSUM"))

        def wload(name, dram, shape, dt):
            t = wp.tile(shape, dt, tag=name)
            nc.scalar.dma_start(t[:], dram[:])
            return t

        statW = wload("statW", statW_d, [128, KT, 3, 128], BF16)
        statH = wload("statH", statH_d, [128, KT, 3, 128], BF16)
        wkW = wload("wkW", wkW_d, [128, KT, 2, 128], BF16)
        wkH = wload("wkH", wkH_d, [128, KT, 2, 128], BF16)
        fusA = wload("fusA", fusA_d, [128, KT, 2, 128], BF16)
        fusB = wload("fusB", fusB_d, [128, KT, 2, 128], BF16)

        bias_sb = wp.tile([128, 7, 2], F32, tag="biases")
        nc.scalar.dma_start(bias_sb[:], bias_d[:].transpose([2, 0, 1]))
        zb = wp.tile([128, 1], F32, tag="zb")
        nc.vector.memset(zb[:], 0.0)

        scr = phv.tile([128, 64, 64], BF16, tag="scr")

        def bap(i, ct):
            return bias_sb[:, i, ct].unsqueeze(1)

        def load_x(b):
            xbt = pxb.tile([128, KT, HW], BF16, tag="xb")
            for half in range(2):
                hs = bass.ts(half, HW // 2)
                nc.sync.dma_start(xbt[:, :, hs], xbf_d[b][:, :, hs].transpose([1, 0, 2]))
            return xbt

        def p1(src, T, gate, stat, bq0, bqrow, bv0, bvrow):
            bq = zb[:] if bq0 else bap(bqrow, 0)
            for j in range(NCH):
                sl = bass.ts(j, CH)
                ps_q = pq.tile([128, CH], F32, tag="q")
                ps_v = pvf.tile([128, 2 * CH], F32, tag="vf")
                for kt in range(KT):
                    st, sp = kt == 0, kt == KT - 1
                    rhs = src[:, kt, sl]
                    nc.tensor.matmul(ps_q[:], stat[:, kt, 2, :], rhs, start=st, stop=sp)
                    nc.tensor.matmul(ps_v[:, 0:CH], stat[:, kt, 0, :], rhs, start=st, stop=sp)
                    nc.tensor.matmul(ps_v[:, CH:], stat[:, kt, 1, :], rhs, start=st, stop=sp)
                nc.scalar.activation(T[:, 0, sl], ps_q[:], AF.Exp, bias=bq)
                if bv0:
                    nc.scalar.activation(
                        gate[:, :, sl], ps_v[:].rearrange("p (c n) -> p c n", c=2),
                        AF.Tanh, bias=zb[:], scale=0.5,
                    )
                else:
                    for ct in range(2):
                        nc.scalar.activation(
                            gate[:, ct, sl], ps_v[:, bass.ts(ct, CH)],
                            AF.Tanh, bias=bap(bvrow, ct), scale=0.5,
                        )
                eng = nc.gpsimd if (GP_U_MASK >> j) & 1 else nc.vector
                eb = T[:, 0, sl].unsqueeze(1).broadcast_to([128, 2, CH])
                eng.tensor_tensor(T[:, 1:3, sl], src[:, :, sl], eb, op=ALU.mult)

        def rtree(T, acc, axis_w):
            # image-level halving tree per T row; every level writes a fresh
            # scratch region (fresh-dest 3D [rows, n] shapes hit fast DVE modes)
            for r in range(3):
                v = T[:, r, :].rearrange("p (a b) -> p a b", b=64)
                tt = nc.vector.tensor_tensor
                if axis_w:
                    tt(scr[:, :, 0:32], v[:, :, 0:32], v[:, :, 32:64], op=ALU.add)
                    tt(scr[:, :, 32:48], scr[:, :, 0:16], scr[:, :, 16:32], op=ALU.add)
                    tt(scr[:, :, 48:56], scr[:, :, 32:40], scr[:, :, 40:48], op=ALU.add)
                    tt(scr[:, :, 56:60], scr[:, :, 48:52], scr[:, :, 52:56], op=ALU.add)
                    tt(scr[:, :, 60:62], scr[:, :, 56:58], scr[:, :, 58:60], op=ALU.add)
                    tt(acc[:, r, :], scr[:, :, 60], scr[:, :, 61], op=ALU.add)
                else:
                    tt(scr[:, 0:32, :], v[:, 0:32, :], v[:, 32:64, :], op=ALU.add)
                    tt(scr[:, 32:48, :], scr[:, 0:16, :], scr[:, 16:32, :], op=ALU.add)
                    tt(scr[:, 48:56, :], scr[:, 32:40, :], scr[:, 40:48, :], op=ALU.add)
                    tt(scr[:, 56:60, :], scr[:, 48:52, :], scr[:, 52:56, :], op=ALU.add)
                    tt(scr[:, 60:62, :], scr[:, 56:58, :], scr[:, 58:60, :], op=ALU.add)
                    tt(acc[:, r, :], scr[:, 60, :], scr[:, 61, :], op=ALU.add)

        def p2(acc, wk, bk0, bkrow, tag):
            R = psm.tile([128, 64], F32, tag=f"R{tag}")
            nc.vector.reciprocal(R[:], acc[:, 0, :])
            xn = psm.tile([128, 2, 64], BF16, tag=f"xn{tag}")
            nc.vector.tensor_tensor(
                xn[:], acc[:, 1:3, :], R[:].unsqueeze(1).broadcast_to([128, 2, 64]), op=ALU.mult
            )
            cns = []
            for mt in range(2):
                ps_c = pq.tile([128, 64], F32, tag="q")
                for ct in range(2):
                    nc.tensor.matmul(ps_c[:], wk[:, ct, mt, :], xn[:, ct, :], start=ct == 0, stop=ct == 1)
                cn = psm.tile([128, 64], BF16, tag=f"cn{tag}{mt}")
                if bk0:
                    nc.vector.tensor_scalar_mul(cn[:], ps_c[:], 0.5)
                else:
                    nc.vector.tensor_scalar(cn[:], ps_c[:], 0.5, bap(bkrow, mt), op0=ALU.mult, op1=ALU.add)
                cns.append(cn)
            return cns

        def p3W(xbt, gate, cns, xeff):
            for j in range(NCH):
                sl = bass.ts(j, CH)
                g2 = pch.tile([128, 2, GRP, 64], BF16, tag="g2w")
                for ct in range(2):
                    cb = cns[ct][:, bass.ts(j, GRP)].unsqueeze(2).broadcast_to([128, GRP, 64])
                    nc.vector.scalar_tensor_tensor(
                        g2[:, ct], gate[:, ct, sl].rearrange("p (a r) -> p a r", r=64),
                        1.0, cb, op0=ALU.add, op1=ALU.mult,
                    )
                eng = nc.gpsimd if (GP_XE_MASK >> j) & 1 else nc.vector
                eng.tensor_tensor(
                    xeff[:, :, sl], xbt[:, :, sl],
                    g2[:].rearrange("p c a r -> p c (a r)"), op=ALU.add,
                )

        def p3H(b, xeff, gate, cns):
            ydst = y_d[b].rearrange("(m p) n -> p m n", p=128)
            for j in range(NCH):
                sl = bass.ts(j, CH)
                g2 = pch.tile([128, 2, GRP, 64], BF16, tag="g2h")
                for ct in range(2):
                    cb = cns[ct][:].unsqueeze(1).broadcast_to([128, GRP, 64])
                    nc.vector.scalar_tensor_tensor(
                        g2[:, ct], gate[:, ct, sl].rearrange("p (a r) -> p a r", r=64),
                        1.0, cb, op0=ALU.add, op1=ALU.mult,
                    )
                ps_f = pvf.tile([128, 2 * CH], F32, tag="vf")
                g2f = g2[:].rearrange("p c a r -> p c (a r)")
                for mt in range(2):
                    half = ps_f[:, bass.ts(mt, CH)]
                    nc.tensor.matmul(half, fusA[:, 0, mt, :], xeff[:, 0, sl], start=True, stop=False)
                    nc.tensor.matmul(half, fusA[:, 1, mt, :], xeff[:, 1, sl], start=False, stop=False)
                    nc.tensor.matmul(half, fusB[:, 0, mt, :], g2f[:, 0], start=False, stop=False)
                    nc.tensor.matmul(half, fusB[:, 1, mt, :], g2f[:, 1], start=False, stop=True)
                y_t = pyv.tile([128, 2, CH], BF16, tag="y")
                if by0:
                    nc.scalar.activation(
                        y_t[:], ps_f[:].rearrange("p (m n) -> p m n", m=2), AF.Copy
                    )
                else:
                    for mt in range(2):
                        nc.scalar.activation(
                            y_t[:, mt, :], ps_f[:, bass.ts(mt, CH)],
                            AF.Identity, bias=bap(6, mt),
                        )
                nc.sync.dma_start(ydst[:, :, sl], y_t[:])

        # ---- schedule: 2 images, stage phases interleaved ----
        xb0 = load_x(0)
        xb1 = load_x(1)

        TW0 = pT.tile([128, 3, HW], BF16, tag="T")
        gW0 = pg.tile([128, 2, HW], BF16, tag="gate")
        p1(xb0[:], TW0, gW0, statW, bqW0, 4, bvW0, 0)

        TW1 = pT.tile([128, 3, HW], BF16, tag="T")
        gW1 = pg.tile([128, 2, HW], BF16, tag="gate")
        p1(xb1[:], TW1, gW1, statW, bqW0, 4, bvW0, 0)

        aW0 = pacc.tile([128, 3, 64], F32, tag="acc")
        rtree(TW0, aW0, True)
        cnsW0 = p2(aW0, wkW, bkW0, 1, "W0")
        xe0 = pxe.tile([128, KT, HW], BF16, tag="xe")
        p3W(xb0, gW0, cnsW0, xe0)

        aW1 = pacc.tile([128, 3, 64], F32, tag="acc")
        rtree(TW1, aW1, True)
        cnsW1 = p2(aW1, wkW, bkW0, 1, "W1")

        TH0 = pT.tile([128, 3, HW], BF16, tag="T")
        gH0 = pg.tile([128, 2, HW], BF16, tag="gate")
        p1(xe0[:], TH0, gH0, statH, bqH0, 5, bvH0, 2)

        xe1 = pxe.tile([128, KT, HW], BF16, tag="xe")
        p3W(xb1, gW1, cnsW1, xe1)

        aH0 = pacc.tile([128, 3, 64], F32, tag="acc")
        rtree(TH0, aH0, False)
        cnsH0 = p2(aH0, wkH, bkH0, 3, "H0")

        TH1 = pT.tile([128, 3, HW], BF16, tag="T")
        gH1 = pg.tile([128, 2, HW], BF16, tag="gate")
        p1(xe1[:], TH1, gH1, statH, bqH0, 5, bvH0, 2)

        p3H(0, xe0, gH0, cnsH0)

        aH1 = pacc.tile([128, 3, 64], F32, tag="acc")
        rtree(TH1, aH1, False)
        cnsH1 = p2(aH1, wkH, bkH0, 3, "H1")

        p3H(1, xe1, gH1, cnsH1)

    nc.compile()
    return nc


def _stat_np(qkv_w):
    wq = qkv_w[0]
    wk = qkv_w[1 : 1 + C]
    wv = qkv_w[1 + C :]
    stat = np.empty((128, KT, 3, 128), np.float64)
    wkt = np.empty((128, KT, 2, 128), np.float64)
    for kt in range(KT):
        cs = slice(kt * 128, (kt + 1) * 128)
        stat[:, kt, 0, :] = wv[0:128, cs].T
        stat[:, kt, 1, :] = wv[128:256, cs].T
        stat[:, kt, 2, :] = np.repeat(wq[cs][:, None], 128, axis=1)
        wkt[:, kt, 0, :] = wk[0:128, cs].T
        wkt[:, kt, 1, :] = wk[128:256, cs].T
    return stat, wkt


def _fus_np(fw):
    fus = np.empty((128, KT, 2, 128), np.float64)
    for kt in range(KT):
        cs = slice(kt * 128, (kt + 1) * 128)
        fus[:, kt, 0, :] = fw[0:128, cs].T
        fus[:, kt, 1, :] = fw[128:256, cs].T
    return fus


def kernel(x, qkvW_w, qkvW_b, qkvH_w, qkvH_b, fusW_w, fusW_b, fusH_w, fusH_b):
    global LAST_RESULTS
    f64 = np.float64
    x = np.asarray(x, np.float32)
    qW = np.asarray(qkvW_w, f64)
    bW = np.asarray(qkvW_b, f64)
    qH = np.asarray(qkvH_w, f64)
    bH = np.asarray(qkvH_b, f64)
    fW = np.asarray(fusW_w, f64)
    fWb = np.asarray(fusW_b, f64)
    fH = np.asarray(fusH_w, f64)
    fHb = np.asarray(fusH_b, f64)

    # stage fold: stage H consumes xeffW directly
    qHf = qH @ fW
    bHf = qH @ fWb + bH
    Wff = fH @ fW
    b_y = fH @ fWb + fHb

    statW, wkW = _stat_np(qW)
    statH, wkH = _stat_np(qHf)
    fusA = _fus_np(Wff)
    fusB = _fus_np(fH)

    tobf = lambda a: np.ascontiguousarray(a.astype(np.float32).astype(NPBF))
    statW16 = tobf(statW)
    statH16 = tobf(statH)
    wkW16 = tobf(wkW)
    wkH16 = tobf(wkH)
    fusA16 = tobf(fusA)
    fusB16 = tobf(fusB)

    bqW, bkW, bvW = bW[0], bW[1 : 1 + C], bW[1 + C :]
    bqH, bkH, bvH = bHf[0], bHf[1 : 1 + C], bHf[1 + C :]
    biases = np.stack(
        [
            (0.5 * bvW).reshape(2, 128),
            (0.5 * bkW).reshape(2, 128),
            (0.5 * bvH).reshape(2, 128),
            (0.5 * bkH).reshape(2, 128),
            np.full((2, 128), bqW),
            np.full((2, 128), bqH),
            b_y.reshape(2, 128),
        ]
    ).astype(np.float32)

    flags = tuple(not np.any(a) for a in (bvW, bkW, bqW, bvH, bkH, bqH, b_y))
    if flags not in _BUILD_CACHE:
        _BUILD_CACHE[flags] = _build(flags)
    nc = _BUILD_CACHE[flags]

    x4 = x.reshape(B, KT, 128, HW)
    xb = np.ascontiguousarray(x4.astype(NPBF))
    in_maps = []
    for core in range(NCORES):
        bs = slice(core * BPC, (core + 1) * BPC)
        in_maps.append(
            {
                "xbf": xb[bs],
                "statW": statW16,
                "statH": statH16,
                "wkW": wkW16,
                "wkH": wkH16,
                "fusA": fusA16,
                "fusB": fusB16,
                "biases": biases,
            }
        )

    res = run_bass_kernel_spmd(nc, in_maps, list(range(NCORES)))
    LAST_RESULTS = res
    y = np.concatenate([r["y"] for r in res.results], axis=0)
    return np.ascontiguousarray(y.astype(np.float32).reshape(B, C, H, W))


# revision 15
# speedup vs baseline: 1.1370x; 1.1370x over previous
"""Trainium2 Bass kernel for nn_DualAxisAggAttn (dual-axis aggregation attention).

Reference semantics per batch image x[C=256, H=64, W=64], twice (W axis then H axis):
  qkv = conv1x1(x) -> {q:[1], k:[C], v:[C]};  s = softmax_axis(q)
  ctx[c,a] = sum_r k*s;  out = x + sigmoid(v) * ctx_bcast;  y = conv1x1(out)

Distribution: data-parallel over batch (16 images -> 2 per NeuronCore x 8 cores).

v3 structure (vs the 129us baseline):
  - STAGE FOLD: stage H is linear in y_W before each nonlinearity, so the
    stage-W fusion conv folds into stage-H weights host-side:
      qkvH' = qkvH @ WfW,  Wff = WfH @ WfW,  biases folded likewise.
    Stage H consumes xeffW = x + gW*ctxW directly; the stage-W fusion
    matmul (1/3 of all PE work) and its PSUM evictions vanish.
  - key-path linearity: ctx = Wk @ (sum_r x*E) / S (key conv after reduction).
  - all matmuls bf16 (fp8 DoubleRow measured at the same col/cycle rate as
    bf16 on HW, so it only costs precision).
  - reductions: image-level halving trees with every level writing a fresh
    scratch region (in-place/strided variants measured 2x; fresh-dest
    [64,n]-shaped levels hit the 4x DVE mode).
  - elementwise load split across engines: u-mult and xeff-combine chunks
    alternate DVE <-> GpSimd(Pool) queue; g2 (scalar_tensor_tensor, 1x,
    DVE-only op) stays on DVE; psum evictions all on ACT.
  - sigmoid via tanh ((1+tanh(v/2))/2): exp+tanh share one ACT table set;
    0.5 folds into ctx scale, +1 into the g2 scalar_tensor_tensor.
"""

import numpy as np
import ml_dtypes
from contextlib import ExitStack

import concourse.bass as bass
import concourse.bacc as bacc
import concourse.tile as tile
import concourse.mybir as mybir
from concourse.bass_utils import run_bass_kernel_spmd

F32 = mybir.dt.float32
BF16 = mybir.dt.bfloat16
AF = mybir.ActivationFunctionType
ALU = mybir.AluOpType
AX = mybir.AxisListType
NPBF = ml_dtypes.bfloat16

B, C, H, W = 16, 256, 64, 64
HW = H * W
NCORES = 8
BPC = B // NCORES
KT = 2
CH = 512
NCH = HW // CH
GRP = CH // 64

# Pool-queue offload masks (chunk j goes to GpSimd when bit set).
# Measured: ANY Pool-engine elementwise traffic slows concurrent DVE ops by
# ~44% (shared SBUF port) — keep these 0.
GP_U_MASK = 0x00
GP_XE_MASK = 0x00

_BUILD_CACHE = {}
LAST_RESULTS = None


def _build(flags):
    bvW0, bkW0, bqW0, bvH0, bkH0, bqH0, by0 = flags
    nc = bacc.Bacc(trn_type="TRN2", target_bir_lowering=False, debug=False)

    xbf_d = nc.dram_tensor("xbf", [BPC, KT, 128, HW], BF16, kind="ExternalInput").ap()
    statW_d = nc.dram_tensor("statW", [128, KT, 3, 128], BF16, kind="ExternalInput").ap()
    statH_d = nc.dram_tensor("statH", [128, KT, 3, 128], BF16, kind="ExternalInput").ap()
    wkW_d = nc.dram_tensor("wkW", [128, KT, 2, 128], BF16, kind="ExternalInput").ap()
    wkH_d = nc.dram_tensor("wkH", [128, KT, 2, 128], BF16, kind="ExternalInput").ap()
    fusA_d = nc.dram_tensor("fusA", [128, KT, 2, 128], BF16, kind="ExternalInput").ap()
    fusB_d = nc.dram_tensor("fusB", [128, KT, 2, 128], BF16, kind="ExternalInput").ap()
    bias_d = nc.dram_tensor("biases", [7, 2, 128], F32, kind="ExternalInput").ap()
    y_d = nc.dram_tensor("y", [BPC, C, HW], BF16, kind="ExternalOutput").ap()

    with tile.TileContext(nc) as tc, ExitStack() as ctx:
        wp = ctx.enter_context(tc.tile_pool(name="weights", bufs=1))
        pxb = ctx.enter_context(tc.tile_pool(name="xb", bufs=2))
        pxe = ctx.enter_context(tc.tile_pool(name="xe", bufs=2))
        pT = ctx.enter_context(tc.tile_pool(name="T", bufs=2))
        pg = ctx.enter_context(tc.tile_pool(name="gate", bufs=2))
        pacc = ctx.enter_context(tc.tile_pool(name="acc", bufs=2))
        psm = ctx.enter_context(tc.tile_pool(name="small", bufs=1))
        pch = ctx.enter_context(tc.tile_pool(name="chunk", bufs=2))
        phv = ctx.enter_context(tc.tile_pool(name="hv", bufs=1))
        pyv = ctx.enter_context(tc.tile_pool(name="yev", bufs=2))
        pq = ctx.enter_context(tc.tile_pool(name="psq", bufs=2, space="PSUM"))
        pvf = ctx.enter_context(tc.tile_pool(name="psvf", bufs=3, space="PSUM"))

        def wload(name, dram, shape, dt):
            t = wp.tile(shape, dt, tag=name)
            nc.scalar.dma_start(t[:], dram[:])
            return t

        statW = wload("statW", statW_d, [128, KT, 3, 128], BF16)
        statH = wload("statH", statH_d, [128, KT, 3, 128], BF16)
        wkW = wload("wkW", wkW_d, [128, KT, 2, 128], BF16)
        wkH = wload("wkH", wkH_d, [128, KT, 2, 128], BF16)
        fusA = wload("fusA", fusA_d, [128, KT, 2, 128], BF16)
        fusB = wload("fusB", fusB_d, [128, KT, 2, 128], BF16)

        bias_sb = wp.tile([128, 7, 2], F32, tag="biases")
        nc.scalar.dma_start(bias_sb[:], bias_d[:].transpose([2, 0, 1]))
        zb = wp.tile([128, 1], F32, tag="zb")
        nc.vector.memset(zb[:], 0.0)

        scr = phv.tile([128, 64, 64], BF16, tag="scr")

        def bap(i, ct):
            return bias_sb[:, i, ct].unsqueeze(1)

        def load_x(b):
            xbt = pxb.tile([128, KT, HW], BF16, tag="xb")
            for half in range(2):
                hs = bass.ts(half, HW // 2)
                nc.sync.dma_start(xbt[:, :, hs], xbf_d[b][:, :, hs].transpose([1, 0, 2]))
            return xbt

        def p1(src, T, gate, stat, bq0, bqrow, bv0, bvrow):
            bq = zb[:] if bq0 else bap(bqrow, 0)
            for j in range(NCH):
                sl = bass.ts(j, CH)
                ps_q = pq.tile([128, CH], F32, tag="q")
                ps_v = pvf.tile([128, 2 * CH], F32, tag="vf")
                for kt in range(KT):
                    st, sp = kt == 0, kt == KT - 1
                    rhs = src[:, kt, sl]
                    nc.tensor.matmul(ps_q[:], stat[:, kt, 2, :], rhs, start=st, stop=sp)
                    nc.tensor.matmul(ps_v[:, 0:CH], stat[:, kt, 0, :], rhs, start=st, stop=sp)
                    nc.tensor.matmul(ps_v[:, CH:], stat[:, kt, 1, :], rhs, start=st, stop=sp)
                nc.scalar.activation(T[:, 0, sl], ps_q[:], AF.Exp, bias=bq)
                if bv0:
                    nc.scalar.activation(
                        gate[:, :, sl], ps_v[:].rearrange("p (c n) -> p c n", c=2),
                        AF.Tanh, bias=zb[:], scale=0.5,
                    )
                else:
                    for ct in range(2):
                        nc.scalar.activation(
                            gate[:, ct, sl], ps_v[:, bass.ts(ct, CH)],
                            AF.Tanh, bias=bap(bvrow, ct), scale=0.5,
                        )
                eng = nc.gpsimd if (GP_U_MASK >> j) & 1 else nc.vector
                eb = T[:, 0, sl].unsqueeze(1).broadcast_to([128, 2, CH])
                eng.tensor_tensor(T[:, 1:3, sl], src[:, :, sl], eb, op=ALU.mult)

        def rtree(T, acc, axis_w):
            # image-level halving tree per T row; every level writes a fresh
            # scratch region (fresh-dest 3D [rows, n] shapes hit fast DVE modes)
            for r in range(3):
                v = T[:, r, :].rearrange("p (a b) -> p a b", b=64)
                tt = nc.vector.tensor_tensor
                if axis_w:
                    tt(scr[:, :, 0:32], v[:, :, 0:32], v[:, :, 32:64], op=ALU.add)
                    tt(scr[:, :, 32:48], scr[:, :, 0:16], scr[:, :, 16:32], op=ALU.add)
                    tt(scr[:, :, 48:56], scr[:, :, 32:40], scr[:, :, 40:48], op=ALU.add)
                    tt(scr[:, :, 56:60], scr[:, :, 48:52], scr[:, :, 52:56], op=ALU.add)
                    tt(scr[:, :, 60:62], scr[:, :, 56:58], scr[:, :, 58:60], op=ALU.add)
                    tt(acc[:, r, :], scr[:, :, 60], scr[:, :, 61], op=ALU.add)
                else:
                    tt(scr[:, 0:32, :], v[:, 0:32, :], v[:, 32:64, :], op=ALU.add)
                    tt(scr[:, 32:48, :], scr[:, 0:16, :], scr[:, 16:32, :], op=ALU.add)
                    tt(scr[:, 48:56, :], scr[:, 32:40, :], scr[:, 40:48, :], op=ALU.add)
                    tt(scr[:, 56:60, :], scr[:, 48:52, :], scr[:, 52:56, :], op=ALU.add)
                    tt(scr[:, 60:62, :], scr[:, 56:58, :], scr[:, 58:60, :], op=ALU.add)
                    tt(acc[:, r, :], scr[:, 60, :], scr[:, 61, :], op=ALU.add)

        def p2(acc, wk, bk0, bkrow, tag):
            R = psm.tile([128, 64], F32, tag=f"R{tag}")
            nc.vector.reciprocal(R[:], acc[:, 0, :])
            xn = psm.tile([128, 2, 64], BF16, tag=f"xn{tag}")
            nc.vector.tensor_tensor(
                xn[:], acc[:, 1:3, :], R[:].unsqueeze(1).broadcast_to([128, 2, 64]), op=ALU.mult
            )
            cns = []
            for mt in range(2):
                ps_c = pq.tile([128, 64], F32, tag="q")
                for ct in range(2):
                    nc.tensor.matmul(ps_c[:], wk[:, ct, mt, :], xn[:, ct, :], start=ct == 0, stop=ct == 1)
                cn = psm.tile([128, 64], BF16, tag=f"cn{tag}{mt}")
                if bk0:
                    nc.vector.tensor_scalar_mul(cn[:], ps_c[:], 0.5)
                else:
                    nc.vector.tensor_scalar(cn[:], ps_c[:], 0.5, bap(bkrow, mt), op0=ALU.mult, op1=ALU.add)
                cns.append(cn)
            return cns

        def p3W(xbt, gate, cns, xeff):
            # image-level per-ct g2 + combine in [64,64]-shaped 3D ops (the
            # shapes that hit the fast DVE ADD mode for the combine)
            for ct in range(2):
                g2 = pch.tile([128, 64, 64], BF16, tag="g2")
                cb = cns[ct][:].unsqueeze(2).broadcast_to([128, 64, 64])
                nc.vector.scalar_tensor_tensor(
                    g2[:], gate[:, ct, :].rearrange("p (a r) -> p a r", r=64),
                    1.0, cb, op0=ALU.add, op1=ALU.mult,
                )
                nc.vector.tensor_tensor(
                    xeff[:, ct, :].rearrange("p (a r) -> p a r", r=64),
                    xbt[:, ct, :].rearrange("p (a r) -> p a r", r=64),
                    g2[:], op=ALU.add,
                )

        def p3H(b, xeff, gate, cns):
            ydst = y_d[b].rearrange("(m p) n -> p m n", p=128)
            g2s = []
            for ct in range(2):
                g2 = pch.tile([128, 64, 64], BF16, tag="g2")
                cb = cns[ct][:].unsqueeze(1).broadcast_to([128, 64, 64])
                nc.vector.scalar_tensor_tensor(
                    g2[:], gate[:, ct, :].rearrange("p (a r) -> p a r", r=64),
                    1.0, cb, op0=ALU.add, op1=ALU.mult,
                )
                g2s.append(g2)
            for j in range(NCH):
                sl = bass.ts(j, CH)
                gsl = bass.ts(j, GRP)
                ps_f = pvf.tile([128, 2 * CH], F32, tag="vf")
                for mt in range(2):
                    half = ps_f[:, bass.ts(mt, CH)]
                    nc.tensor.matmul(half, fusA[:, 0, mt, :], xeff[:, 0, sl], start=True, stop=False)
                    nc.tensor.matmul(half, fusA[:, 1, mt, :], xeff[:, 1, sl], start=False, stop=False)
                    nc.tensor.matmul(half, fusB[:, 0, mt, :], g2s[0][:, gsl, :], start=False, stop=False)
                    nc.tensor.matmul(half, fusB[:, 1, mt, :], g2s[1][:, gsl, :], start=False, stop=True)
                y_t = pyv.tile([128, 2, CH], BF16, tag="y")
                if by0:
                    nc.scalar.activation(
                        y_t[:], ps_f[:].rearrange("p (m n) -> p m n", m=2), AF.Copy
                    )
                else:
                    for mt in range(2):
                        nc.scalar.activation(
                            y_t[:, mt, :], ps_f[:, bass.ts(mt, CH)],
                            AF.Identity, bias=bap(6, mt),
                        )
                nc.sync.dma_start(ydst[:, :, sl], y_t[:])

        # ---- schedule: 2 images, stage phases interleaved ----
        xb0 = load_x(0)
        xb1 = load_x(1)

        TW0 = pT.tile([128, 3, HW], BF16, tag="T")
        gW0 = pg.tile([128, 2, HW], BF16, tag="gate")
        p1(xb0[:], TW0, gW0, statW, bqW0, 4, bvW0, 0)

        TW1 = pT.tile([128, 3, HW], BF16, tag="T")
        gW1 = pg.tile([128, 2, HW], BF16, tag="gate")
        p1(xb1[:], TW1, gW1, statW, bqW0, 4, bvW0, 0)

        aW0 = pacc.tile([128, 3, 64], F32, tag="acc")
        rtree(TW0, aW0, True)
        cnsW0 = p2(aW0, wkW, bkW0, 1, "W0")
        xe0 = pxe.tile([128, KT, HW], BF16, tag="xe")
        p3W(xb0, gW0, cnsW0, xe0)

        aW1 = pacc.tile([128, 3, 64], F32, tag="acc")
        rtree(TW1, aW1, True)
        cnsW1 = p2(aW1, wkW, bkW0, 1, "W1")

        TH0 = pT.tile([128, 3, HW], BF16, tag="T")
        gH0 = pg.tile([128, 2, HW], BF16, tag="gate")
        p1(xe0[:], TH0, gH0, statH, bqH0, 5, bvH0, 2)

        xe1 = pxe.tile([128, KT, HW], BF16, tag="xe")
        p3W(xb1, gW1, cnsW1, xe1)

        aH0 = pacc.tile([128, 3, 64], F32, tag="acc")
        rtree(TH0, aH0, False)
        cnsH0 = p2(aH0, wkH, bkH0, 3, "H0")

        TH1 = pT.tile([128, 3, HW], BF16, tag="T")
        gH1 = pg.tile([128, 2, HW], BF16, tag="gate")
        p1(xe1[:], TH1, gH1, statH, bqH0, 5, bvH0, 2)

        p3H(0, xe0, gH0, cnsH0)

        aH1 = pacc.tile([128, 3, 64], F32, tag="acc")
        rtree(TH1, aH1, False)
        cnsH1 = p2(aH1, wkH, bkH0, 3, "H1")

        p3H(1, xe1, gH1, cnsH1)

    nc.compile()
    return nc


def _stat_np(qkv_w):
    wq = qkv_w[0]
    wk = qkv_w[1 : 1 + C]
    wv = qkv_w[1 + C :]
    stat = np.empty((128, KT, 3, 128), np.float64)
    wkt = np.empty((128, KT, 2, 128), np.float64)
    for kt in range(KT):
        cs = slice(kt * 128, (kt + 1) * 128)
        stat[:, kt, 0, :] = wv[0:128, cs].T
        stat[:, kt, 1, :] = wv[128:256, cs].T
        stat[:, kt, 2, :] = np.repeat(wq[cs][:, None], 128, axis=1)
        wkt[:, kt, 0, :] = wk[0:128, cs].T
        wkt[:, kt, 1, :] = wk[128:256, cs].T
    return stat, wkt


def _fus_np(fw):
    fus = np.empty((128, KT, 2, 128), np.float64)
    for kt in range(KT):
        cs = slice(kt * 128, (kt + 1) * 128)
        fus[:, kt, 0, :] = fw[0:128, cs].T
        fus[:, kt, 1, :] = fw[128:256, cs].T
    return fus


def kernel(x, qkvW_w, qkvW_b, qkvH_w, qkvH_b, fusW_w, fusW_b, fusH_w, fusH_b):
    global LAST_RESULTS
    f64 = np.float64
    x = np.asarray(x, np.float32)
    qW = np.asarray(qkvW_w, f64)
    bW = np.asarray(qkvW_b, f64)
    qH = np.asarray(qkvH_w, f64)
    bH = np.asarray(qkvH_b, f64)
    fW = np.asarray(fusW_w, f64)
    fWb = np.asarray(fusW_b, f64)
    fH = np.asarray(fusH_w, f64)
    fHb = np.asarray(fusH_b, f64)

    # stage fold: stage H consumes xeffW directly
    qHf = qH @ fW
    bHf = qH @ fWb + bH
    Wff = fH @ fW
    b_y = fH @ fWb + fHb

    statW, wkW = _stat_np(qW)
    statH, wkH = _stat_np(qHf)
    fusA = _fus_np(Wff)
    fusB = _fus_np(fH)

    tobf = lambda a: np.ascontiguousarray(a.astype(np.float32).astype(NPBF))
    statW16 = tobf(statW)
    statH16 = tobf(statH)
    wkW16 = tobf(wkW)
    wkH16 = tobf(wkH)
    fusA16 = tobf(fusA)
    fusB16 = tobf(fusB)

    bqW, bkW, bvW = bW[0], bW[1 : 1 + C], bW[1 + C :]
    bqH, bkH, bvH = bHf[0], bHf[1 : 1 + C], bHf[1 + C :]
    biases = np.stack(
        [
            (0.5 * bvW).reshape(2, 128),
            (0.5 * bkW).reshape(2, 128),
            (0.5 * bvH).reshape(2, 128),
            (0.5 * bkH).reshape(2, 128),
            np.full((2, 128), bqW),
            np.full((2, 128), bqH),
            b_y.reshape(2, 128),
        ]
    ).astype(np.float32)

    flags = tuple(not np.any(a) for a in (bvW, bkW, bqW, bvH, bkH, bqH, b_y))
    if flags not in _BUILD_CACHE:
        _BUILD_CACHE[flags] = _build(flags)
    nc = _BUILD_CACHE[flags]

    x4 = x.reshape(B, KT, 128, HW)
    xb = np.ascontiguousarray(x4.astype(NPBF))
    in_maps = []
    for core in range(NCORES):
        bs = slice(core * BPC, (core + 1) * BPC)
        in_maps.append(
            {
                "xbf": xb[bs],
                "statW": statW16,
                "statH": statH16,
                "wkW": wkW16,
                "wkH": wkH16,
                "fusA": fusA16,
                "fusB": fusB16,
                "biases": biases,
            }
        )

    res = run_bass_kernel_spmd(nc, in_maps, list(range(NCORES)))
    LAST_RESULTS = res
    y = np.concatenate([r["y"] for r in res.results], axis=0)
    return np.ascontiguousarray(y.astype(np.float32).reshape(B, C, H, W))


# revision 19
# speedup vs baseline: 1.1453x; 1.0073x over previous
"""Trainium2 Bass kernel for nn_DualAxisAggAttn (dual-axis aggregation attention).

Reference semantics per batch image x[C=256, H=64, W=64], twice (W axis then H axis):
  qkv = conv1x1(x) -> {q:[1], k:[C], v:[C]};  s = softmax_axis(q)
  ctx[c,a] = sum_r k*s;  out = x + sigmoid(v) * ctx_bcast;  y = conv1x1(out)

Distribution: data-parallel over batch (16 images -> 2 per NeuronCore x 8 cores).

v3 structure (vs the 129us baseline):
  - STAGE FOLD: stage H is linear in y_W before each nonlinearity, so the
    stage-W fusion conv folds into stage-H weights host-side:
      qkvH' = qkvH @ WfW,  Wff = WfH @ WfW,  biases folded likewise.
    Stage H consumes xeffW = x + gW*ctxW directly; the stage-W fusion
    matmul (1/3 of all PE work) and its PSUM evictions vanish.
  - key-path linearity: ctx = Wk @ (sum_r x*E) / S (key conv after reduction).
  - all matmuls bf16 (fp8 DoubleRow measured at the same col/cycle rate as
    bf16 on HW, so it only costs precision).
  - reductions: image-level halving trees with every level writing a fresh
    scratch region (in-place/strided variants measured 2x; fresh-dest
    [64,n]-shaped levels hit the 4x DVE mode).
  - elementwise load split across engines: u-mult and xeff-combine chunks
    alternate DVE <-> GpSimd(Pool) queue; g2 (scalar_tensor_tensor, 1x,
    DVE-only op) stays on DVE; psum evictions all on ACT.
  - sigmoid via tanh ((1+tanh(v/2))/2): exp+tanh share one ACT table set;
    0.5 folds into ctx scale, +1 into the g2 scalar_tensor_tensor.
"""

import numpy as np
import ml_dtypes
from contextlib import ExitStack

import concourse.bass as bass
import concourse.bacc as bacc
import concourse.tile as tile
import concourse.mybir as mybir
from concourse.bass_utils import run_bass_kernel_spmd

F32 = mybir.dt.float32
BF16 = mybir.dt.bfloat16
AF = mybir.ActivationFunctionType
ALU = mybir.AluOpType
AX = mybir.AxisListType
NPBF = ml_dtypes.bfloat16

B, C, H, W = 16, 256, 64, 64
HW = H * W
NCORES = 8
BPC = B // NCORES
KT = 2
CH = 512
NCH = HW // CH
GRP = CH // 64

# Pool-queue offload masks (chunk j goes to GpSimd when bit set).
# Measured: ANY Pool-engine elementwise traffic slows concurrent DVE ops by
# ~44% (shared SBUF port) — keep these 0.
GP_U_MASK = 0x00
GP_XE_MASK = 0x00

_BUILD_CACHE = {}
LAST_RESULTS = None


def _build(flags):
    bvW0, bkW0, bqW0, bvH0, bkH0, bqH0, by0 = flags
    nc = bacc.Bacc(trn_type="TRN2", target_bir_lowering=False, debug=False)

    xbf_d = nc.dram_tensor("xbf", [BPC, KT, 128, HW], BF16, kind="ExternalInput").ap()
    statW_d = nc.dram_tensor("statW", [128, KT, 3, 128], BF16, kind="ExternalInput").ap()
    statH_d = nc.dram_tensor("statH", [128, KT, 3, 128], BF16, kind="ExternalInput").ap()
    wkW_d = nc.dram_tensor("wkW", [128, KT, 2, 128], BF16, kind="ExternalInput").ap()
    wkH_d = nc.dram_tensor("wkH", [128, KT, 2, 128], BF16, kind="ExternalInput").ap()
    fusA_d = nc.dram_tensor("fusA", [128, KT, 2, 128], BF16, kind="ExternalInput").ap()
    fusB_d = nc.dram_tensor("fusB", [128, KT, 2, 128], BF16, kind="ExternalInput").ap()
    bias_d = nc.dram_tensor("biases", [7, 2, 128], F32, kind="ExternalInput").ap()
    y_d = nc.dram_tensor("y", [BPC, C, HW], BF16, kind="ExternalOutput").ap()

    with tile.TileContext(nc) as tc, ExitStack() as ctx:
        wp = ctx.enter_context(tc.tile_pool(name="weights", bufs=1))
        pxb = ctx.enter_context(tc.tile_pool(name="xb", bufs=2))
        pxe = ctx.enter_context(tc.tile_pool(name="xe", bufs=2))
        pT = ctx.enter_context(tc.tile_pool(name="T", bufs=2))
        pg = ctx.enter_context(tc.tile_pool(name="gate", bufs=2))
        pacc = ctx.enter_context(tc.tile_pool(name="acc", bufs=2))
        psm = ctx.enter_context(tc.tile_pool(name="small", bufs=1))
        pch = ctx.enter_context(tc.tile_pool(name="chunk", bufs=2))
        phv = ctx.enter_context(tc.tile_pool(name="hv", bufs=1))
        pyv = ctx.enter_context(tc.tile_pool(name="yev", bufs=2))
        pq = ctx.enter_context(tc.tile_pool(name="psq", bufs=2, space="PSUM"))
        pvf = ctx.enter_context(tc.tile_pool(name="psvf", bufs=3, space="PSUM"))

        def wload(name, dram, shape, dt):
            t = wp.tile(shape, dt, tag=name)
            nc.scalar.dma_start(t[:], dram[:])
            return t

        statW = wload("statW", statW_d, [128, KT, 3, 128], BF16)
        statH = wload("statH", statH_d, [128, KT, 3, 128], BF16)
        wkW = wload("wkW", wkW_d, [128, KT, 2, 128], BF16)
        wkH = wload("wkH", wkH_d, [128, KT, 2, 128], BF16)
        fusA = wload("fusA", fusA_d, [128, KT, 2, 128], BF16)
        fusB = wload("fusB", fusB_d, [128, KT, 2, 128], BF16)

        bias_sb = wp.tile([128, 7, 2], F32, tag="biases")
        nc.scalar.dma_start(bias_sb[:], bias_d[:].transpose([2, 0, 1]))
        zb = wp.tile([128, 1], F32, tag="zb")
        nc.vector.memset(zb[:], 0.0)

        scr = phv.tile([128, 64, 64], BF16, tag="scr")

        def bap(i, ct):
            return bias_sb[:, i, ct].unsqueeze(1)

        def load_x(b):
            xbt = pxb.tile([128, KT, HW], BF16, tag="xb")
            for half in range(2):
                hs = bass.ts(half, HW // 2)
                nc.sync.dma_start(xbt[:, :, hs], xbf_d[b][:, :, hs].transpose([1, 0, 2]))
            return xbt

        def p1(src, T, gate, stat, bq0, bqrow, bv0, bvrow):
            bq = zb[:] if bq0 else bap(bqrow, 0)
            for j in range(NCH):
                sl = bass.ts(j, CH)
                ps_q = pq.tile([128, CH], F32, tag="q")
                ps_v = pvf.tile([128, 2 * CH], F32, tag="vf")
                for kt in range(KT):
                    st, sp = kt == 0, kt == KT - 1
                    rhs = src[:, kt, sl]
                    nc.tensor.matmul(ps_q[:], stat[:, kt, 2, :], rhs, start=st, stop=sp)
                    nc.tensor.matmul(ps_v[:, 0:CH], stat[:, kt, 0, :], rhs, start=st, stop=sp)
                    nc.tensor.matmul(ps_v[:, CH:], stat[:, kt, 1, :], rhs, start=st, stop=sp)
                nc.scalar.activation(T[:, 0, sl], ps_q[:], AF.Exp, bias=bq)
                if bv0:
                    nc.scalar.activation(
                        gate[:, :, sl], ps_v[:].rearrange("p (c n) -> p c n", c=2),
                        AF.Tanh, bias=zb[:], scale=0.5,
                    )
                else:
                    for ct in range(2):
                        nc.scalar.activation(
                            gate[:, ct, sl], ps_v[:, bass.ts(ct, CH)],
                            AF.Tanh, bias=bap(bvrow, ct), scale=0.5,
                        )
                # u-mult per half-image in [32,64]-row shapes (the fast-mode
                # DVE shape), emitted once chunks 0-3 / 4-7 are done
                if j % 4 == 3:
                    hs = bass.ts(j // 4, HW // 2)
                    for ct in range(2):
                        nc.vector.tensor_tensor(
                            T[:, 1 + ct, hs].rearrange("p (a r) -> p a r", r=64),
                            src[:, ct, hs].rearrange("p (a r) -> p a r", r=64),
                            T[:, 0, hs].rearrange("p (a r) -> p a r", r=64),
                            op=ALU.mult,
                        )

        def rtree(T, acc, axis_w):
            # image-level halving tree per T row; every level writes a fresh
            # scratch region (fresh-dest 3D [rows, n] shapes hit fast DVE modes)
            for r in range(3):
                v = T[:, r, :].rearrange("p (a b) -> p a b", b=64)
                tt = nc.vector.tensor_tensor
                if axis_w:
                    tt(scr[:, :, 0:32], v[:, :, 0:32], v[:, :, 32:64], op=ALU.add)
                    tt(scr[:, :, 32:48], scr[:, :, 0:16], scr[:, :, 16:32], op=ALU.add)
                    tt(scr[:, :, 48:56], scr[:, :, 32:40], scr[:, :, 40:48], op=ALU.add)
                    tt(scr[:, :, 56:60], scr[:, :, 48:52], scr[:, :, 52:56], op=ALU.add)
                    tt(scr[:, :, 60:62], scr[:, :, 56:58], scr[:, :, 58:60], op=ALU.add)
                    tt(acc[:, r, :], scr[:, :, 60], scr[:, :, 61], op=ALU.add)
                else:
                    tt(scr[:, 0:32, :], v[:, 0:32, :], v[:, 32:64, :], op=ALU.add)
                    tt(scr[:, 32:48, :], scr[:, 0:16, :], scr[:, 16:32, :], op=ALU.add)
                    tt(scr[:, 48:56, :], scr[:, 32:40, :], scr[:, 40:48, :], op=ALU.add)
                    tt(scr[:, 56:60, :], scr[:, 48:52, :], scr[:, 52:56, :], op=ALU.add)
                    tt(scr[:, 60:62, :], scr[:, 56:58, :], scr[:, 58:60, :], op=ALU.add)
                    tt(acc[:, r, :], scr[:, 60, :], scr[:, 61, :], op=ALU.add)

        def p2(acc, wk, bk0, bkrow, tag):
            R = psm.tile([128, 64], F32, tag=f"R{tag}")
            nc.vector.reciprocal(R[:], acc[:, 0, :])
            xn = psm.tile([128, 2, 64], BF16, tag=f"xn{tag}")
            nc.vector.tensor_tensor(
                xn[:], acc[:, 1:3, :], R[:].unsqueeze(1).broadcast_to([128, 2, 64]), op=ALU.mult
            )
            cns = []
            for mt in range(2):
                ps_c = pq.tile([128, 64], F32, tag="q")
                for ct in range(2):
                    nc.tensor.matmul(ps_c[:], wk[:, ct, mt, :], xn[:, ct, :], start=ct == 0, stop=ct == 1)
                cn = psm.tile([128, 64], BF16, tag=f"cn{tag}{mt}")
                if bk0:
                    nc.vector.tensor_scalar_mul(cn[:], ps_c[:], 0.5)
                else:
                    nc.vector.tensor_scalar(cn[:], ps_c[:], 0.5, bap(bkrow, mt), op0=ALU.mult, op1=ALU.add)
                cns.append(cn)
            return cns

        def p3W(xbt, gate, cns, xeff):
            # per-(half, ct) g2 + combine in [32,64]-shaped 3D ops (the
            # shape class that hits the fast DVE modes); half granularity
            # also pipelines into the stage-H consumers
            for half in range(2):
                hs = bass.ts(half, HW // 2)
                ha = bass.ts(half, 32)
                for ct in range(2):
                    g2 = pch.tile([128, 64, 64], BF16, tag="g2")
                    cb = cns[ct][:, ha].unsqueeze(2).broadcast_to([128, 32, 64])
                    nc.vector.scalar_tensor_tensor(
                        g2[:, ha], gate[:, ct, hs].rearrange("p (a r) -> p a r", r=64),
                        1.0, cb, op0=ALU.add, op1=ALU.mult,
                    )
                    nc.vector.tensor_tensor(
                        xeff[:, ct, hs].rearrange("p (a r) -> p a r", r=64),
                        xbt[:, ct, hs].rearrange("p (a r) -> p a r", r=64),
                        g2[:, ha], op=ALU.add,
                    )

        def p3H(b, xeff, gate, cns):
            ydst = y_d[b].rearrange("(m p) n -> p m n", p=128)
            g2s = []
            for ct in range(2):
                g2 = pch.tile([128, 64, 64], BF16, tag="g2")
                for half in range(2):
                    hs = bass.ts(half, HW // 2)
                    ha = bass.ts(half, 32)
                    cb = cns[ct][:].unsqueeze(1).broadcast_to([128, 32, 64])
                    nc.vector.scalar_tensor_tensor(
                        g2[:, ha], gate[:, ct, hs].rearrange("p (a r) -> p a r", r=64),
                        1.0, cb, op0=ALU.add, op1=ALU.mult,
                    )
                g2s.append(g2)
            for j in range(NCH):
                sl = bass.ts(j, CH)
                gsl = bass.ts(j, GRP)
                ps_f = pvf.tile([128, 2 * CH], F32, tag="vf")
                for mt in range(2):
                    half = ps_f[:, bass.ts(mt, CH)]
                    nc.tensor.matmul(half, fusA[:, 0, mt, :], xeff[:, 0, sl], start=True, stop=False)
                    nc.tensor.matmul(half, fusA[:, 1, mt, :], xeff[:, 1, sl], start=False, stop=False)
                    nc.tensor.matmul(half, fusB[:, 0, mt, :], g2s[0][:, gsl, :], start=False, stop=False)
                    nc.tensor.matmul(half, fusB[:, 1, mt, :], g2s[1][:, gsl, :], start=False, stop=True)
                y_t = pyv.tile([128, 2, CH], BF16, tag="y")
                if by0:
                    nc.scalar.activation(
                        y_t[:], ps_f[:].rearrange("p (m n) -> p m n", m=2), AF.Copy
                    )
                else:
                    for mt in range(2):
                        nc.scalar.activation(
                            y_t[:, mt, :], ps_f[:, bass.ts(mt, CH)],
                            AF.Identity, bias=bap(6, mt),
                        )
                nc.sync.dma_start(ydst[:, :, sl], y_t[:])

        # ---- schedule: 2 images, stage phases interleaved ----
        xb0 = load_x(0)
        xb1 = load_x(1)

        TW0 = pT.tile([128, 3, HW], BF16, tag="T")
        gW0 = pg.tile([128, 2, HW], BF16, tag="gate")
        p1(xb0[:], TW0, gW0, statW, bqW0, 4, bvW0, 0)

        TW1 = pT.tile([128, 3, HW], BF16, tag="T")
        gW1 = pg.tile([128, 2, HW], BF16, tag="gate")
        p1(xb1[:], TW1, gW1, statW, bqW0, 4, bvW0, 0)

        aW0 = pacc.tile([128, 3, 64], F32, tag="acc")
        rtree(TW0, aW0, True)
        cnsW0 = p2(aW0, wkW, bkW0, 1, "W0")
        xe0 = pxe.tile([128, KT, HW], BF16, tag="xe")
        p3W(xb0, gW0, cnsW0, xe0)

        aW1 = pacc.tile([128, 3, 64], F32, tag="acc")
        rtree(TW1, aW1, True)
        cnsW1 = p2(aW1, wkW, bkW0, 1, "W1")

        TH0 = pT.tile([128, 3, HW], BF16, tag="T")
        gH0 = pg.tile([128, 2, HW], BF16, tag="gate")
        p1(xe0[:], TH0, gH0, statH, bqH0, 5, bvH0, 2)

        xe1 = pxe.tile([128, KT, HW], BF16, tag="xe")
        p3W(xb1, gW1, cnsW1, xe1)

        aH0 = pacc.tile([128, 3, 64], F32, tag="acc")
        rtree(TH0, aH0, False)
        cnsH0 = p2(aH0, wkH, bkH0, 3, "H0")

        TH1 = pT.tile([128, 3, HW], BF16, tag="T")
        gH1 = pg.tile([128, 2, HW], BF16, tag="gate")
        p1(xe1[:], TH1, gH1, statH, bqH0, 5, bvH0, 2)

        p3H(0, xe0, gH0, cnsH0)

        aH1 = pacc.tile([128, 3, 64], F32, tag="acc")
        rtree(TH1, aH1, False)
        cnsH1 = p2(aH1, wkH, bkH0, 3, "H1")

        p3H(1, xe1, gH1, cnsH1)

    nc.compile()
    return nc


def _stat_np(qkv_w):
    wq = qkv_w[0]
    wk = qkv_w[1 : 1 + C]
    wv = qkv_w[1 + C :]
    stat = np.empty((128, KT, 3, 128), np.float64)
    wkt = np.empty((128, KT, 2, 128), np.float64)
    for kt in range(KT):
        cs = slice(kt * 128, (kt + 1) * 128)
        stat[:, kt, 0, :] = wv[0:128, cs].T
        stat[:, kt, 1, :] = wv[128:256, cs].T
        stat[:, kt, 2, :] = np.repeat(wq[cs][:, None], 128, axis=1)
        wkt[:, kt, 0, :] = wk[0:128, cs].T
        wkt[:, kt, 1, :] = wk[128:256, cs].T
    return stat, wkt


def _fus_np(fw):
    fus = np.empty((128, KT, 2, 128), np.float64)
    for kt in range(KT):
        cs = slice(kt * 128, (kt + 1) * 128)
        fus[:, kt, 0, :] = fw[0:128, cs].T
        fus[:, kt, 1, :] = fw[128:256, cs].T
    return fus


def kernel(x, qkvW_w, qkvW_b, qkvH_w, qkvH_b, fusW_w, fusW_b, fusH_w, fusH_b):
    global LAST_RESULTS
    f64 = np.float64
    x = np.asarray(x, np.float32)
    qW = np.asarray(qkvW_w, f64)
    bW = np.asarray(qkvW_b, f64)
    qH = np.asarray(qkvH_w, f64)
    bH = np.asarray(qkvH_b, f64)
    fW = np.asarray(fusW_w, f64)
    fWb = np.asarray(fusW_b, f64)
    fH = np.asarray(fusH_w, f64)
    fHb = np.asarray(fusH_b, f64)

    # stage fold: stage H consumes xeffW directly
    qHf = qH @ fW
    bHf = qH @ fWb + bH
    Wff = fH @ fW
    b_y = fH @ fWb + fHb

    statW, wkW = _stat_np(qW)
    statH, wkH = _stat_np(qHf)
    fusA = _fus_np(Wff)
    fusB = _fus_np(fH)

    tobf = lambda a: np.ascontiguousarray(a.astype(np.float32).astype(NPBF))
    statW16 = tobf(statW)
    statH16 = tobf(statH)
    wkW16 = tobf(wkW)
    wkH16 = tobf(wkH)
    fusA16 = tobf(fusA)
    fusB16 = tobf(fusB)

    bqW, bkW, bvW = bW[0], bW[1 : 1 + C], bW[1 + C :]
    bqH, bkH, bvH = bHf[0], bHf[1 : 1 + C], bHf[1 + C :]
    biases = np.stack(
        [
            (0.5 * bvW).reshape(2, 128),
            (0.5 * bkW).reshape(2, 128),
            (0.5 * bvH).reshape(2, 128),
            (0.5 * bkH).reshape(2, 128),
            np.full((2, 128), bqW),
            np.full((2, 128), bqH),
            b_y.reshape(2, 128),
        ]
    ).astype(np.float32)

    flags = tuple(not np.any(a) for a in (bvW, bkW, bqW, bvH, bkH, bqH, b_y))
    if flags not in _BUILD_CACHE:
        _BUILD_CACHE[flags] = _build(flags)
    nc = _BUILD_CACHE[flags]

    x4 = x.reshape(B, KT, 128, HW)
    xb = np.ascontiguousarray(x4.astype(NPBF))
    in_maps = []
    for core in range(NCORES):
        bs = slice(core * BPC, (core + 1) * BPC)
        in_maps.append(
            {
                "xbf": xb[bs],
                "statW": statW16,
                "statH": statH16,
                "wkW": wkW16,
                "wkH": wkH16,
                "fusA": fusA16,
                "fusB": fusB16,
                "biases": biases,
            }
        )

    res = run_bass_kernel_spmd(nc, in_maps, list(range(NCORES)))
    LAST_RESULTS = res
    y = np.concatenate([r["y"] for r in res.results], axis=0)
    return np.ascontiguousarray(y.astype(np.float32).reshape(B, C, H, W))


# revision 22
# speedup vs baseline: 1.3189x; 1.1516x over previous
"""Trainium2 Bass kernel for nn_DualAxisAggAttn (dual-axis aggregation attention).

Reference semantics per batch image x[C=256, H=64, W=64], twice (W axis then H axis):
  qkv = conv1x1(x) -> {q:[1], k:[C], v:[C]};  s = softmax_axis(q)
  ctx[c,a] = sum_r k*s;  out = x + sigmoid(v) * ctx_bcast;  y = conv1x1(out)

Distribution: data-parallel over batch (16 images -> 2 per NeuronCore x 8 cores).

Key optimizations:
  - key-path linearity: ctx = Wk @ (sum_r x*E) / S -- the key 1x1 conv moves
    AFTER the softmax-weighted reduction (N=4096 -> N=64 moving columns).
  - combine folded into the fusion matmul: ps = Wf@x + Wf@g2 (psum accum),
    so `out = x + g2` is never materialized.
  - query row replicated 128x in its m-tile -> exp(q) lands partition-broadcast.
  - sigmoid via tanh ((1+tanh(v/2))/2): exp+tanh+copy share ONE ACT table set;
    the +1 is applied in-place on the gate (4x tensor_scalar), the 0.5 folds
    into the softmax normalizer and k-bias.
  - all matmuls bf16 (host pre-casts inputs; measured rel err ~3e-3 vs fp32).
  - reductions inner-contiguous (stage W halve+reduce, stage H contiguous
    binary tree over h); no strided elementwise ops.
  - per-engine instruction streams are FIFO, so batch-stage PHASES are
    interleaved at emission time to keep the PE fed during reduce chains.
  - GPSIMD does nothing (it contends with DVE for the shared SBUF port).
"""

import numpy as np
import ml_dtypes
from contextlib import ExitStack

import concourse.bass as bass
import concourse.bacc as bacc
import concourse.tile as tile
import concourse.mybir as mybir
from concourse.bass_utils import run_bass_kernel_spmd

F32 = mybir.dt.float32
BF16 = mybir.dt.bfloat16
AF = mybir.ActivationFunctionType
ALU = mybir.AluOpType
AX = mybir.AxisListType
NPBF = ml_dtypes.bfloat16

B, C, H, W = 16, 256, 64, 64
HW = H * W
NCORES = 8
BPC = B // NCORES
KT = 2
CH = 512
NCH = HW // CH
GRP = CH // 64

_BUILD_CACHE = {}
LAST_RESULTS = None


class _Stage:
    """Emits one attention stage (one batch) in three phases."""

    def __init__(self, nc, pools, axis_w, srcs, stat, wk, fus, bias, dst_evict):
        self.nc, self.axis_w = nc, axis_w
        self.scr = pools[-1]
        self.srcs, self.stat, self.wk, self.fus = srcs, stat, wk, fus
        self.bias, self.dst_evict = bias, dst_evict
        (self.pbig, self.pgate, self.pchunk, self.pctx, self.pq, self.pv, self.pf) = pools[:7]

    def p1_alloc(self):
        self.E = self.pbig.tile([128, HW], BF16, tag="E")
        self.gate = self.pgate.tile([128, 2, HW], BF16, tag="gate")
        self.u = self.pbig.tile([128, 2, HW], BF16, tag="u")

    def p1_chunk(self, j):
        nc, srcs, stat, bias = self.nc, self.srcs, self.stat, self.bias
        E, gate, u = self.E, self.gate, self.u
        bv2 = bias.get("bv2")
        if True:
            sl = bass.ts(j, CH)
            ps_q = self.pq.tile([128, CH], F32, tag="q")
            ps_v = self.pv.tile([128, 2 * CH], F32, tag="vf")
            for kt in range(KT):
                st, sp = kt == 0, kt == KT - 1
                rhs = srcs[j][:, kt, :]
                nc.tensor.matmul(ps_q[:], stat[:, kt, 2, :], rhs, start=st, stop=sp)
                nc.tensor.matmul(ps_v[:, 0:CH], stat[:, kt, 0, :], rhs, start=st, stop=sp)
                nc.tensor.matmul(ps_v[:, CH:], stat[:, kt, 1, :], rhs, start=st, stop=sp)
            nc.scalar.activation(E[:, sl], ps_q[:], AF.Exp, bias=bias["zb"])
            if bv2 is None:
                nc.scalar.activation(
                    gate[:, :, sl], ps_v[:].rearrange("p (c n) -> p c n", c=2),
                    AF.Tanh, bias=bias["zb"], scale=0.5,
                )
            else:
                nc.scalar.activation(gate[:, 0, sl], ps_v[:, 0:CH], AF.Tanh, bias=bv2[0], scale=0.5)
                nc.scalar.activation(gate[:, 1, sl], ps_v[:, CH:], AF.Tanh, bias=bv2[1], scale=0.5)
            eb = E[:, sl].unsqueeze(1).broadcast_to([128, 2, CH])
            nc.vector.tensor_tensor(u[:, :, sl], srcs[j][:, :, :], eb, op=ALU.mult)
            if not self.axis_w:
                nc.vector.tensor_scalar_add(gate[:, :, sl], gate[:, :, sl], 1.0)

    def _reduce64(self, flat, tag):
        # halving tree over one [64,64]-viewed row; every level writes a
        # fresh region of the shared scratch (these shapes hit the fast
        # DVE ADD modes; in-place or flat variants run 2x slower)
        nc, pctx, scr = self.nc, self.pctx, self.scr
        out = pctx.tile([128, 64], F32, tag=f"red_{tag}")
        v = flat.rearrange("p (a b) -> p a b", b=64)
        tt = nc.vector.tensor_tensor
        if self.axis_w:
            tt(scr[:, :, 0:32], v[:, :, 0:32], v[:, :, 32:64], op=ALU.add)
            tt(scr[:, :, 32:48], scr[:, :, 0:16], scr[:, :, 16:32], op=ALU.add)
            tt(scr[:, :, 48:56], scr[:, :, 32:40], scr[:, :, 40:48], op=ALU.add)
            tt(scr[:, :, 56:60], scr[:, :, 48:52], scr[:, :, 52:56], op=ALU.add)
            tt(scr[:, :, 60:62], scr[:, :, 56:58], scr[:, :, 58:60], op=ALU.add)
            tt(out[:], scr[:, :, 60], scr[:, :, 61], op=ALU.add)
        else:
            tt(scr[:, 0:32, :], v[:, 0:32, :], v[:, 32:64, :], op=ALU.add)
            tt(scr[:, 32:48, :], scr[:, 0:16, :], scr[:, 16:32, :], op=ALU.add)
            tt(scr[:, 48:56, :], scr[:, 32:40, :], scr[:, 40:48, :], op=ALU.add)
            tt(scr[:, 56:60, :], scr[:, 48:52, :], scr[:, 52:56, :], op=ALU.add)
            tt(scr[:, 60:62, :], scr[:, 56:58, :], scr[:, 58:60, :], op=ALU.add)
            tt(out[:], scr[:, 60, :], scr[:, 61, :], op=ALU.add)
        return out

    def p2(self):
        nc, pctx, bias = self.nc, self.pctx, self.bias
        S = self._reduce64(self.E[:], "S")
        R = pctx.tile([128, 64], F32, tag="R")
        nc.vector.reciprocal(R[:], S[:])
        xen = []
        xes = [self._reduce64(self.u[:, ct, :], f"xe{ct}") for ct in range(2)]
        for ct in range(2):
            xn = pctx.tile([128, 64], BF16, tag=f"xn{ct}")
            nc.vector.tensor_tensor(xn[:], xes[ct], R[:], op=ALU.mult)
            xen.append(xn)
        self.ctxs = []
        bk2 = bias.get("bk2")
        for mt in range(2):
            ps_c = self.pq.tile([128, 64], F32, tag="q")
            for ct in range(2):
                nc.tensor.matmul(ps_c[:], self.wk[:, ct, mt, :], xen[ct][:], start=ct == 0, stop=ct == 1)
            cn = pctx.tile([128, 64], BF16, tag=f"cn{mt}")
            if bk2 is None:
                nc.vector.tensor_scalar_mul(cn[:], ps_c[:], 0.5)
            else:
                nc.vector.tensor_scalar(cn[:], ps_c[:], 0.5, bk2[mt], op0=ALU.mult, op1=ALU.add)
            self.ctxs.append(cn)
        # g2 per (half, ct) in [32,64]-shaped ops; for the H axis the +1 was
        # already folded into the gate during p1 (tensor_scalar_add), so a
        # plain 2x-mode multiply suffices; the W axis needs the 1x stt
        self.g2s = []
        for ct in range(2):
            g2 = self.pchunk.tile([128, 64, 64], BF16, tag="g2big", bufs=2)
            for half in range(2):
                ha = bass.ts(half, 32)
                hs = bass.ts(half, HW // 2)
                gv = self.gate[:, ct, hs].rearrange("p (a r) -> p a r", r=64)
                if self.axis_w:
                    cb = self.ctxs[ct][:, ha].unsqueeze(2).broadcast_to([128, 32, 64])
                    nc.vector.scalar_tensor_tensor(g2[:, ha], gv, 1.0, cb, op0=ALU.add, op1=ALU.mult)
                else:
                    cb = self.ctxs[ct][:].unsqueeze(1).broadcast_to([128, 32, 64])
                    nc.vector.tensor_tensor(g2[:, ha], gv, cb, op=ALU.mult)
            self.g2s.append(g2)

    def p3_chunk(self, j):
        nc, srcs, fus = self.nc, self.srcs, self.fus
        if True:
            sl = bass.ts(j, CH)
            gsl = bass.ts(j, GRP)
            ps_f = self.pf.tile([128, 2 * CH], F32, tag="vf")
            for mt in range(2):
                half = ps_f[:, bass.ts(mt, CH)]
                nc.tensor.matmul(half, fus[:, 0, mt, :], srcs[j][:, 0, :], start=True, stop=False)
                nc.tensor.matmul(half, fus[:, 1, mt, :], srcs[j][:, 1, :], start=False, stop=False)
                nc.tensor.matmul(half, fus[:, 0, mt, :], self.g2s[0][:, gsl, :], start=False, stop=False)
                nc.tensor.matmul(half, fus[:, 1, mt, :], self.g2s[1][:, gsl, :], start=False, stop=True)
            self.dst_evict(j, ps_f)


def _build(flags):
    bvW0, bkW0, bvH0, bkH0, bfW0, bfH0 = flags
    nc = bacc.Bacc(trn_type="TRN2", target_bir_lowering=False, debug=False)

    x_d = nc.dram_tensor("x", [BPC, C, HW], BF16, kind="ExternalInput").ap()
    statW_d = nc.dram_tensor("statW", [128, KT, 3, 128], BF16, kind="ExternalInput").ap()
    statH_d = nc.dram_tensor("statH", [128, KT, 3, 128], BF16, kind="ExternalInput").ap()
    wkW_d = nc.dram_tensor("wkW", [128, KT, 2, 128], BF16, kind="ExternalInput").ap()
    wkH_d = nc.dram_tensor("wkH", [128, KT, 2, 128], BF16, kind="ExternalInput").ap()
    fusW_d = nc.dram_tensor("fusW", [128, KT, 2, 128], BF16, kind="ExternalInput").ap()
    fusH_d = nc.dram_tensor("fusH", [128, KT, 2, 128], BF16, kind="ExternalInput").ap()
    bias_d = nc.dram_tensor("biases", [6, 2, 128], F32, kind="ExternalInput").ap()
    y_d = nc.dram_tensor("y", [BPC, C, HW], BF16, kind="ExternalOutput").ap()

    with tile.TileContext(nc) as tc, ExitStack() as ctx:
        wp = ctx.enter_context(tc.tile_pool(name="weights", bufs=1))
        xbp = ctx.enter_context(tc.tile_pool(name="xbf", bufs=18))
        xwp = ctx.enter_context(tc.tile_pool(name="xw", bufs=16))
        pbig = ctx.enter_context(tc.tile_pool(name="big", bufs=2))
        pgate = ctx.enter_context(tc.tile_pool(name="gate", bufs=2))
        pchunk = ctx.enter_context(tc.tile_pool(name="chunk", bufs=3))
        pctx = ctx.enter_context(tc.tile_pool(name="ctx", bufs=3))
        phv = ctx.enter_context(tc.tile_pool(name="hv", bufs=2))
        yp = ctx.enter_context(tc.tile_pool(name="yev", bufs=3))
        pq = ctx.enter_context(tc.tile_pool(name="psq", bufs=2, space="PSUM"))
        pvf = ctx.enter_context(tc.tile_pool(name="psvf", bufs=3, space="PSUM"))
        def wload(name, dram, shape, dt):
            t = wp.tile(shape, dt, tag=name)
            nc.scalar.dma_start(t[:], dram[:])
            return t

        statW = wload("statW", statW_d, [128, KT, 3, 128], BF16)
        statH = wload("statH", statH_d, [128, KT, 3, 128], BF16)
        wkW = wload("wkW", wkW_d, [128, KT, 2, 128], BF16)
        wkH = wload("wkH", wkH_d, [128, KT, 2, 128], BF16)
        fusW = wload("fusW", fusW_d, [128, KT, 2, 128], BF16)
        fusH = wload("fusH", fusH_d, [128, KT, 2, 128], BF16)

        bias_sb = wp.tile([128, 6, 2], F32, tag="biases")
        nc.scalar.dma_start(bias_sb[:], bias_d[:].transpose([2, 0, 1]))
        zb = wp.tile([128, 1], F32, tag="zb")
        nc.vector.memset(zb[:], 0.0)

        scr = phv.tile([128, 64, 64], BF16, tag="scr", bufs=1)
        pools = (pbig, pgate, pchunk, pctx, pq, pvf, pvf, scr)

        def bap(i, ct):
            return bias_sb[:, i, ct].unsqueeze(1)

        biasW = {
            "bv2": None if bvW0 else [bap(0, ct) for ct in range(2)],
            "bk2": None if bkW0 else [bap(1, ct) for ct in range(2)],
            "zb": zb[:],
        }
        biasH = {
            "bv2": None if bvH0 else [bap(2, ct) for ct in range(2)],
            "bk2": None if bkH0 else [bap(3, ct) for ct in range(2)],
            "zb": zb[:],
        }

        def load_x(b):
            xcs = []
            for j in range(NCH):
                xc = xbp.tile([128, KT, CH], BF16, tag="xc")
                for kt in range(KT):
                    nc.sync.dma_start(xc[:, kt, :], x_d[b, bass.ts(kt, 128), bass.ts(j, CH)])
                xcs.append(xc[:])
            return xcs

        def make_W(b, xcs):
            xw_tiles = [None] * NCH

            def evW(j, ps_f):
                xw = xwp.tile([128, KT, CH], BF16, tag="xw")
                xw_tiles[j] = xw[:]
                if bfW0:
                    nc.scalar.activation(xw[:], ps_f[:].rearrange("p (c n) -> p c n", c=2), AF.Copy)
                else:
                    for ct in range(2):
                        nc.scalar.activation(
                            xw[:, ct, :], ps_f[:, bass.ts(ct, CH)],
                            AF.Identity, bias=bap(4, ct),
                        )

            st = _Stage(nc, pools, True, xcs, statW, wkW, fusW, biasW, evW)
            st.xw_tiles = xw_tiles
            return st

        def make_H(b, xw_tiles):
            def evH(j, ps_f):
                y_t = yp.tile([128, 2, CH], BF16, tag="y")
                if bfH0:
                    nc.scalar.activation(y_t[:], ps_f[:].rearrange("p (c n) -> p c n", c=2), AF.Copy)
                else:
                    for ct in range(2):
                        nc.scalar.activation(
                            y_t[:, ct, :], ps_f[:, bass.ts(ct, CH)],
                            AF.Identity, bias=bap(5, ct),
                        )
                nc.sync.dma_start(
                    y_d[b].rearrange("(c p) n -> p c n", p=128)[:, :, bass.ts(j, CH)],
                    y_t[:],
                )

            return _Stage(nc, pools, False, xw_tiles, statH, wkH, fusH, biasH, evH)

        # interleaved phase schedule: chunk-level alternation keeps every
        # engine's FIFO stream fed during the other phase's stalls
        def run_p1(st):
            st.p1_alloc()
            for j in range(NCH):
                st.p1_chunk(j)

        def run_p3(st):
            for j in range(NCH):
                st.p3_chunk(j)

        x0 = load_x(0)
        x1 = load_x(1)
        w0 = make_W(0, x0)
        w1 = make_W(1, x1)
        run_p1(w0)
        run_p1(w1)
        w0.p2()
        run_p3(w0)
        h0 = make_H(0, w0.xw_tiles)
        w1.p2()
        run_p1(h0)
        run_p3(w1)
        h1 = make_H(1, w1.xw_tiles)
        h0.p2()
        run_p1(h1)
        run_p3(h0)
        h1.p2()
        run_p3(h1)

    nc.compile()
    return nc


def _prep(qkv_w, fus_w):
    wq = qkv_w[0]
    wk = qkv_w[1 : 1 + C]
    wv = qkv_w[1 + C :]
    stat = np.empty((128, KT, 3, 128), np.float32)
    wkt = np.empty((128, KT, 2, 128), np.float32)
    fus = np.empty((128, KT, 2, 128), np.float32)
    for kt in range(KT):
        cs = slice(kt * 128, (kt + 1) * 128)
        stat[:, kt, 0, :] = wv[0:128, cs].T
        stat[:, kt, 1, :] = wv[128:256, cs].T
        stat[:, kt, 2, :] = np.repeat(wq[cs][:, None], 128, axis=1)
        wkt[:, kt, 0, :] = wk[0:128, cs].T
        wkt[:, kt, 1, :] = wk[128:256, cs].T
        fus[:, kt, 0, :] = fus_w[0:128, cs].T
        fus[:, kt, 1, :] = fus_w[128:256, cs].T
    tobf = lambda a: np.ascontiguousarray(a.astype(NPBF))
    return tobf(stat), tobf(wkt), tobf(fus)


def kernel(x, qkvW_w, qkvW_b, qkvH_w, qkvH_b, fusW_w, fusW_b, fusH_w, fusH_b):
    global LAST_RESULTS
    x = np.asarray(x, np.float32)
    qkvW_w = np.asarray(qkvW_w, np.float32)
    qkvW_b = np.asarray(qkvW_b, np.float32)
    qkvH_w = np.asarray(qkvH_w, np.float32)
    qkvH_b = np.asarray(qkvH_b, np.float32)
    fusW_w = np.asarray(fusW_w, np.float32)
    fusW_b = np.asarray(fusW_b, np.float32)
    fusH_w = np.asarray(fusH_w, np.float32)
    fusH_b = np.asarray(fusH_b, np.float32)

    statW, wkW, fusW = _prep(qkvW_w, fusW_w)
    statH, wkH, fusH = _prep(qkvH_w, fusH_w)

    bkW = qkvW_b[1 : 1 + C]
    bvW = qkvW_b[1 + C :]
    bkH = qkvH_b[1 : 1 + C]
    bvH = qkvH_b[1 + C :]
    biases = np.stack(
        [0.5 * bvW.reshape(2, 128),
         0.5 * bkW.reshape(2, 128),
         0.5 * bvH.reshape(2, 128),
         0.5 * bkH.reshape(2, 128),
         fusW_b.reshape(2, 128),
         fusH_b.reshape(2, 128)]
    ).astype(np.float32)

    flags = (
        not bvW.any(), not bkW.any(), not bvH.any(), not bkH.any(),
        not fusW_b.any(), not fusH_b.any(),
    )
    if flags not in _BUILD_CACHE:
        _BUILD_CACHE[flags] = _build(flags)
    nc = _BUILD_CACHE[flags]

    xbf = np.ascontiguousarray(x.reshape(B, C, HW).astype(NPBF))
    in_maps = []
    for core in range(NCORES):
        in_maps.append({
            "x": xbf[core * BPC : (core + 1) * BPC],
            "statW": statW, "statH": statH,
            "wkW": wkW, "wkH": wkH, "fusW": fusW, "fusH": fusH,
            "biases": biases,
        })

    res = run_bass_kernel_spmd(nc, in_maps, list(range(NCORES)))
    LAST_RESULTS = res
    y = np.concatenate([r["y"] for r in res.results], axis=0)
    return y.astype(np.float32).reshape(B, C, H, W)



# revision 23
# speedup vs baseline: 1.3692x; 1.0382x over previous
"""Trainium2 Bass kernel for nn_DualAxisAggAttn (dual-axis aggregation attention).

Reference semantics per batch image x[C=256, H=64, W=64], twice (W axis then H axis):
  qkv = conv1x1(x) -> {q:[1], k:[C], v:[C]};  s = softmax_axis(q)
  ctx[c,a] = sum_r k*s;  out = x + sigmoid(v) * ctx_bcast;  y = conv1x1(out)

Distribution: data-parallel over batch (16 images -> 2 per NeuronCore x 8 cores).

Key optimizations:
  - key-path linearity: ctx = Wk @ (sum_r x*E) / S -- the key 1x1 conv moves
    AFTER the softmax-weighted reduction (N=4096 -> N=64 moving columns).
  - combine folded into the fusion matmul: ps = Wf@x + Wf@g2 (psum accum),
    so `out = x + g2` is never materialized.
  - query row replicated 128x in its m-tile -> exp(q) lands partition-broadcast.
  - sigmoid via tanh ((1+tanh(v/2))/2): exp+tanh+copy share ONE ACT table set;
    the +1 is applied in-place on the gate (4x tensor_scalar), the 0.5 folds
    into the softmax normalizer and k-bias.
  - all matmuls bf16 (host pre-casts inputs; measured rel err ~3e-3 vs fp32).
  - reductions inner-contiguous (stage W halve+reduce, stage H contiguous
    binary tree over h); no strided elementwise ops.
  - per-engine instruction streams are FIFO, so batch-stage PHASES are
    interleaved at emission time to keep the PE fed during reduce chains.
  - GPSIMD does nothing (it contends with DVE for the shared SBUF port).
"""

import numpy as np
import ml_dtypes
from contextlib import ExitStack

import concourse.bass as bass
import concourse.bacc as bacc
import concourse.tile as tile
import concourse.mybir as mybir
from concourse.bass_utils import run_bass_kernel_spmd

F32 = mybir.dt.float32
BF16 = mybir.dt.bfloat16
AF = mybir.ActivationFunctionType
ALU = mybir.AluOpType
AX = mybir.AxisListType
NPBF = ml_dtypes.bfloat16

B, C, H, W = 16, 256, 64, 64
HW = H * W
NCORES = 8
BPC = B // NCORES
KT = 2
CH = 512
NCH = HW // CH
GRP = CH // 64

_BUILD_CACHE = {}
LAST_RESULTS = None


class _Stage:
    """Emits one attention stage (one batch) in three phases."""

    def __init__(self, nc, pools, axis_w, srcs, stat, wk, fus, bias, dst_evict):
        self.nc, self.axis_w = nc, axis_w
        self.phv = pools[-1]
        self.srcs, self.stat, self.wk, self.fus = srcs, stat, wk, fus
        self.bias, self.dst_evict = bias, dst_evict
        (self.pbig, self.pgate, self.pchunk, self.pctx, self.pq, self.pv, self.pf) = pools[:7]

    def p1_alloc(self):
        self.E = self.pbig.tile([128, HW], BF16, tag="E")
        self.gate = self.pgate.tile([128, 2, HW], BF16, tag="gate")
        self.u = self.pbig.tile([128, 2, HW], BF16, tag="u")

    def p1_chunk(self, j):
        nc, srcs, stat, bias = self.nc, self.srcs, self.stat, self.bias
        E, gate, u = self.E, self.gate, self.u
        bv2 = bias.get("bv2")
        if True:
            sl = bass.ts(j, CH)
            ps_q = self.pq.tile([128, CH], F32, tag="q")
            ps_v = self.pv.tile([128, 2 * CH], F32, tag="vf")
            for kt in range(KT):
                st, sp = kt == 0, kt == KT - 1
                rhs = srcs[j][:, kt, :]
                nc.tensor.matmul(ps_q[:], stat[:, kt, 2, :], rhs, start=st, stop=sp)
                nc.tensor.matmul(ps_v[:, 0:CH], stat[:, kt, 0, :], rhs, start=st, stop=sp)
                nc.tensor.matmul(ps_v[:, CH:], stat[:, kt, 1, :], rhs, start=st, stop=sp)
            nc.scalar.activation(E[:, sl], ps_q[:], AF.Exp, bias=bias["zb"])
            if bv2 is None:
                nc.scalar.activation(
                    gate[:, :, sl], ps_v[:].rearrange("p (c n) -> p c n", c=2),
                    AF.Tanh, bias=bias["zb"], scale=0.5,
                )
            else:
                nc.scalar.activation(gate[:, 0, sl], ps_v[:, 0:CH], AF.Tanh, bias=bv2[0], scale=0.5)
                nc.scalar.activation(gate[:, 1, sl], ps_v[:, CH:], AF.Tanh, bias=bv2[1], scale=0.5)
            eb = E[:, sl].unsqueeze(1).broadcast_to([128, 2, CH])
            nc.vector.tensor_tensor(u[:, :, sl], srcs[j][:, :, :], eb, op=ALU.mult)
            if not self.axis_w:
                nc.vector.tensor_scalar_add(gate[:, :, sl], gate[:, :, sl], 1.0)

    def _reduce64(self, flat, tag):
        nc, pctx = self.nc, self.pctx
        if self.axis_w:
            v3 = flat.rearrange("p (a r) -> p a r", r=64)
            hv = self.phv.tile([128, 64, 32], BF16, tag="hv")
            nc.vector.tensor_tensor(hv[:], v3[:, :, 0:32], v3[:, :, 32:64], op=ALU.add)
            h2 = self.phv.tile([128, 64, 16], BF16, tag="hv2")
            nc.vector.tensor_tensor(h2[:], hv[:, :, 0:16], hv[:, :, 16:32], op=ALU.add)
            nc.vector.tensor_tensor(h2[:, :, 0:8], h2[:, :, 0:8], h2[:, :, 8:16], op=ALU.add)
            nc.vector.tensor_tensor(h2[:, :, 0:4], h2[:, :, 0:4], h2[:, :, 4:8], op=ALU.add)
            nc.vector.tensor_tensor(h2[:, :, 0:2], h2[:, :, 0:2], h2[:, :, 2:4], op=ALU.add)
            out = pctx.tile([128, 64], F32, tag=f"red_{tag}")
            nc.vector.tensor_tensor(out[:], h2[:, :, 0], h2[:, :, 1], op=ALU.add)
        else:
            t = self.phv.tile([128, 2048], BF16, tag="tree2")
            nc.vector.tensor_tensor(t[:], flat[:, 0:2048], flat[:, 2048:4096], op=ALU.add)
            n = 1024
            while n >= 128:
                nc.vector.tensor_tensor(t[:, 0:n], t[:, 0:n], t[:, n : 2 * n], op=ALU.add)
                n //= 2
            out = pctx.tile([128, 64], F32, tag=f"red_{tag}")
            nc.vector.tensor_tensor(out[:], t[:, 0:64], t[:, 64:128], op=ALU.add)
        return out

    def p2(self):
        nc, pctx, bias = self.nc, self.pctx, self.bias
        S = self._reduce64(self.E[:], "S")
        R = pctx.tile([128, 64], F32, tag="R")
        nc.vector.reciprocal(R[:], S[:])
        xen = []
        if self.axis_w:
            xes = [self._reduce64(self.u[:, ct, :], f"xe{ct}") for ct in range(2)]
        else:
            # merged tree over both c-tiles: [128, 2, n] contiguous views
            t = self.phv.tile([128, 2, 2048], BF16, tag="tree2")
            u = self.u
            nc.vector.tensor_tensor(t[:], u[:, :, 0:2048], u[:, :, 2048:4096], op=ALU.add)
            n = 1024
            while n >= 128:
                nc.vector.tensor_tensor(t[:, :, 0:n], t[:, :, 0:n], t[:, :, n:2*n], op=ALU.add)
                n //= 2
            xep = self.pctx.tile([128, 2, 64], F32, tag="xep")
            nc.vector.tensor_tensor(xep[:], t[:, :, 0:64], t[:, :, 64:128], op=ALU.add)
            xes = [xep[:, ct, :] for ct in range(2)]
        for ct in range(2):
            xn = pctx.tile([128, 64], BF16, tag=f"xn{ct}")
            nc.vector.tensor_tensor(xn[:], xes[ct], R[:], op=ALU.mult)
            xen.append(xn)
        self.ctxs = []
        bk2 = bias.get("bk2")
        for mt in range(2):
            ps_c = self.pq.tile([128, 64], F32, tag="q")
            for ct in range(2):
                nc.tensor.matmul(ps_c[:], self.wk[:, ct, mt, :], xen[ct][:], start=ct == 0, stop=ct == 1)
            cn = pctx.tile([128, 64], BF16, tag=f"cn{mt}")
            if bk2 is None:
                nc.vector.tensor_scalar_mul(cn[:], ps_c[:], 0.5)
            else:
                nc.vector.tensor_scalar(cn[:], ps_c[:], 0.5, bk2[mt], op0=ALU.mult, op1=ALU.add)
            self.ctxs.append(cn)

    def p3_chunk(self, j):
        nc, srcs, fus = self.nc, self.srcs, self.fus
        if True:
            sl = bass.ts(j, CH)
            g2s = []
            for ct in range(2):
                if self.axis_w:
                    cb = self.ctxs[ct][:, bass.ts(j, GRP)].unsqueeze(2).broadcast_to([128, GRP, 64])
                else:
                    cb = self.ctxs[ct][:].unsqueeze(1).broadcast_to([128, GRP, 64])
                g2 = self.pchunk.tile([128, GRP, 64], BF16, tag=f"g2_{ct}")
                gv = self.gate[:, ct, sl].rearrange("p (a r) -> p a r", r=64)
                if self.axis_w:
                    nc.vector.scalar_tensor_tensor(g2[:], gv, 1.0, cb, op0=ALU.add, op1=ALU.mult)
                else:
                    nc.vector.tensor_tensor(g2[:], gv, cb, op=ALU.mult)
                g2s.append(g2)
            ps_f = self.pf.tile([128, 2 * CH], F32, tag="vf")
            for mt in range(2):
                half = ps_f[:, bass.ts(mt, CH)]
                nc.tensor.matmul(half, fus[:, 0, mt, :], srcs[j][:, 0, :], start=True, stop=False)
                nc.tensor.matmul(half, fus[:, 1, mt, :], srcs[j][:, 1, :], start=False, stop=False)
                nc.tensor.matmul(half, fus[:, 0, mt, :], g2s[0][:].rearrange("p a r -> p (a r)"), start=False, stop=False)
                nc.tensor.matmul(half, fus[:, 1, mt, :], g2s[1][:].rearrange("p a r -> p (a r)"), start=False, stop=True)
            self.dst_evict(j, ps_f)


def _build(flags):
    bvW0, bkW0, bvH0, bkH0, bfW0, bfH0 = flags
    nc = bacc.Bacc(trn_type="TRN2", target_bir_lowering=False, debug=False)

    x_d = nc.dram_tensor("x", [BPC, C, HW], BF16, kind="ExternalInput").ap()
    statW_d = nc.dram_tensor("statW", [128, KT, 3, 128], BF16, kind="ExternalInput").ap()
    statH_d = nc.dram_tensor("statH", [128, KT, 3, 128], BF16, kind="ExternalInput").ap()
    wkW_d = nc.dram_tensor("wkW", [128, KT, 2, 128], BF16, kind="ExternalInput").ap()
    wkH_d = nc.dram_tensor("wkH", [128, KT, 2, 128], BF16, kind="ExternalInput").ap()
    fusW_d = nc.dram_tensor("fusW", [128, KT, 2, 128], BF16, kind="ExternalInput").ap()
    fusH_d = nc.dram_tensor("fusH", [128, KT, 2, 128], BF16, kind="ExternalInput").ap()
    bias_d = nc.dram_tensor("biases", [6, 2, 128], F32, kind="ExternalInput").ap()
    y_d = nc.dram_tensor("y", [BPC, C, HW], BF16, kind="ExternalOutput").ap()

    with tile.TileContext(nc) as tc, ExitStack() as ctx:
        wp = ctx.enter_context(tc.tile_pool(name="weights", bufs=1))
        xbp = ctx.enter_context(tc.tile_pool(name="xbf", bufs=18))
        xwp = ctx.enter_context(tc.tile_pool(name="xw", bufs=16))
        pbig = ctx.enter_context(tc.tile_pool(name="big", bufs=2))
        pgate = ctx.enter_context(tc.tile_pool(name="gate", bufs=2))
        pchunk = ctx.enter_context(tc.tile_pool(name="chunk", bufs=3))
        pctx = ctx.enter_context(tc.tile_pool(name="ctx", bufs=3))
        phv = ctx.enter_context(tc.tile_pool(name="hv", bufs=2))
        yp = ctx.enter_context(tc.tile_pool(name="yev", bufs=3))
        pq = ctx.enter_context(tc.tile_pool(name="psq", bufs=2, space="PSUM"))
        pvf = ctx.enter_context(tc.tile_pool(name="psvf", bufs=3, space="PSUM"))
        pools = (pbig, pgate, pchunk, pctx, pq, pvf, pvf, phv)

        def wload(name, dram, shape, dt):
            t = wp.tile(shape, dt, tag=name)
            nc.scalar.dma_start(t[:], dram[:])
            return t

        statW = wload("statW", statW_d, [128, KT, 3, 128], BF16)
        statH = wload("statH", statH_d, [128, KT, 3, 128], BF16)
        wkW = wload("wkW", wkW_d, [128, KT, 2, 128], BF16)
        wkH = wload("wkH", wkH_d, [128, KT, 2, 128], BF16)
        fusW = wload("fusW", fusW_d, [128, KT, 2, 128], BF16)
        fusH = wload("fusH", fusH_d, [128, KT, 2, 128], BF16)

        bias_sb = wp.tile([128, 6, 2], F32, tag="biases")
        nc.scalar.dma_start(bias_sb[:], bias_d[:].transpose([2, 0, 1]))
        zb = wp.tile([128, 1], F32, tag="zb")
        nc.vector.memset(zb[:], 0.0)

        def bap(i, ct):
            return bias_sb[:, i, ct].unsqueeze(1)

        biasW = {
            "bv2": None if bvW0 else [bap(0, ct) for ct in range(2)],
            "bk2": None if bkW0 else [bap(1, ct) for ct in range(2)],
            "zb": zb[:],
        }
        biasH = {
            "bv2": None if bvH0 else [bap(2, ct) for ct in range(2)],
            "bk2": None if bkH0 else [bap(3, ct) for ct in range(2)],
            "zb": zb[:],
        }

        def load_x(b):
            xcs = []
            for j in range(NCH):
                xc = xbp.tile([128, KT, CH], BF16, tag="xc")
                for kt in range(KT):
                    nc.sync.dma_start(xc[:, kt, :], x_d[b, bass.ts(kt, 128), bass.ts(j, CH)])
                xcs.append(xc[:])
            return xcs

        def make_W(b, xcs):
            xw_tiles = [None] * NCH

            def evW(j, ps_f):
                xw = xwp.tile([128, KT, CH], BF16, tag="xw")
                xw_tiles[j] = xw[:]
                if bfW0:
                    nc.scalar.activation(xw[:], ps_f[:].rearrange("p (c n) -> p c n", c=2), AF.Copy)
                else:
                    for ct in range(2):
                        nc.scalar.activation(
                            xw[:, ct, :], ps_f[:, bass.ts(ct, CH)],
                            AF.Identity, bias=bap(4, ct),
                        )

            st = _Stage(nc, pools, True, xcs, statW, wkW, fusW, biasW, evW)
            st.xw_tiles = xw_tiles
            return st

        def make_H(b, xw_tiles):
            def evH(j, ps_f):
                y_t = yp.tile([128, 2, CH], BF16, tag="y")
                if bfH0:
                    nc.scalar.activation(y_t[:], ps_f[:].rearrange("p (c n) -> p c n", c=2), AF.Copy)
                else:
                    for ct in range(2):
                        nc.scalar.activation(
                            y_t[:, ct, :], ps_f[:, bass.ts(ct, CH)],
                            AF.Identity, bias=bap(5, ct),
                        )
                nc.sync.dma_start(
                    y_d[b].rearrange("(c p) n -> p c n", p=128)[:, :, bass.ts(j, CH)],
                    y_t[:],
                )

            return _Stage(nc, pools, False, xw_tiles, statH, wkH, fusH, biasH, evH)

        # interleaved phase schedule: chunk-level alternation keeps every
        # engine's FIFO stream fed during the other phase's stalls
        def run_p1(st):
            st.p1_alloc()
            for j in range(NCH):
                st.p1_chunk(j)

        def run_p3(st):
            for j in range(NCH):
                st.p3_chunk(j)

        x0 = load_x(0)
        x1 = load_x(1)
        w0 = make_W(0, x0)
        w1 = make_W(1, x1)
        run_p1(w0)
        run_p1(w1)
        w0.p2()
        run_p3(w0)
        h0 = make_H(0, w0.xw_tiles)
        w1.p2()
        run_p1(h0)
        run_p3(w1)
        h1 = make_H(1, w1.xw_tiles)
        h0.p2()
        run_p1(h1)
        run_p3(h0)
        h1.p2()
        run_p3(h1)

    nc.compile()
    return nc


def _prep(qkv_w, fus_w):
    wq = qkv_w[0]
    wk = qkv_w[1 : 1 + C]
    wv = qkv_w[1 + C :]
    stat = np.empty((128, KT, 3, 128), np.float32)
    wkt = np.empty((128, KT, 2, 128), np.float32)
    fus = np.empty((128, KT, 2, 128), np.float32)
    for kt in range(KT):
        cs = slice(kt * 128, (kt + 1) * 128)
        stat[:, kt, 0, :] = wv[0:128, cs].T
        stat[:, kt, 1, :] = wv[128:256, cs].T
        stat[:, kt, 2, :] = np.repeat(wq[cs][:, None], 128, axis=1)
        wkt[:, kt, 0, :] = wk[0:128, cs].T
        wkt[:, kt, 1, :] = wk[128:256, cs].T
        fus[:, kt, 0, :] = fus_w[0:128, cs].T
        fus[:, kt, 1, :] = fus_w[128:256, cs].T
    tobf = lambda a: np.ascontiguousarray(a.astype(NPBF))
    return tobf(stat), tobf(wkt), tobf(fus)


def kernel(x, qkvW_w, qkvW_b, qkvH_w, qkvH_b, fusW_w, fusW_b, fusH_w, fusH_b):
    global LAST_RESULTS
    x = np.asarray(x, np.float32)
    qkvW_w = np.asarray(qkvW_w, np.float32)
    qkvW_b = np.asarray(qkvW_b, np.float32)
    qkvH_w = np.asarray(qkvH_w, np.float32)
    qkvH_b = np.asarray(qkvH_b, np.float32)
    fusW_w = np.asarray(fusW_w, np.float32)
    fusW_b = np.asarray(fusW_b, np.float32)
    fusH_w = np.asarray(fusH_w, np.float32)
    fusH_b = np.asarray(fusH_b, np.float32)

    statW, wkW, fusW = _prep(qkvW_w, fusW_w)
    statH, wkH, fusH = _prep(qkvH_w, fusH_w)

    bkW = qkvW_b[1 : 1 + C]
    bvW = qkvW_b[1 + C :]
    bkH = qkvH_b[1 : 1 + C]
    bvH = qkvH_b[1 + C :]
    biases = np.stack(
        [0.5 * bvW.reshape(2, 128),
         0.5 * bkW.reshape(2, 128),
         0.5 * bvH.reshape(2, 128),
         0.5 * bkH.reshape(2, 128),
         fusW_b.reshape(2, 128),
         fusH_b.reshape(2, 128)]
    ).astype(np.float32)

    flags = (
        not bvW.any(), not bkW.any(), not bvH.any(), not bkH.any(),
        not fusW_b.any(), not fusH_b.any(),
    )
    if flags not in _BUILD_CACHE:
        _BUILD_CACHE[flags] = _build(flags)
    nc = _BUILD_CACHE[flags]

    xbf = np.ascontiguousarray(x.reshape(B, C, HW).astype(NPBF))
    in_maps = []
    for core in range(NCORES):
        in_maps.append({
            "x": xbf[core * BPC : (core + 1) * BPC],
            "statW": statW, "statH": statH,
            "wkW": wkW, "wkH": wkH, "fusW": fusW, "fusH": fusH,
            "biases": biases,
        })

    res = run_bass_kernel_spmd(nc, in_maps, list(range(NCORES)))
    LAST_RESULTS = res
    y = np.concatenate([r["y"] for r in res.results], axis=0)
    return y.astype(np.float32).reshape(B, C, H, W)



# revision 24
# speedup vs baseline: 1.3924x; 1.0169x over previous
"""Trainium2 Bass kernel for nn_DualAxisAggAttn (dual-axis aggregation attention).

Reference semantics per batch image x[C=256, H=64, W=64], twice (W axis then H axis):
  qkv = conv1x1(x) -> {q:[1], k:[C], v:[C]};  s = softmax_axis(q)
  ctx[c,a] = sum_r k*s;  out = x + sigmoid(v) * ctx_bcast;  y = conv1x1(out)

Distribution: data-parallel over batch (16 images -> 2 per NeuronCore x 8 cores).

Key optimizations:
  - key-path linearity: ctx = Wk @ (sum_r x*E) / S -- the key 1x1 conv moves
    AFTER the softmax-weighted reduction (N=4096 -> N=64 moving columns).
  - combine folded into the fusion matmul: ps = Wf@x + Wf@g2 (psum accum),
    so `out = x + g2` is never materialized.
  - query row replicated 128x in its m-tile -> exp(q) lands partition-broadcast.
  - sigmoid via tanh ((1+tanh(v/2))/2): exp+tanh+copy share ONE ACT table set;
    the +1 is applied in-place on the gate (4x tensor_scalar), the 0.5 folds
    into the softmax normalizer and k-bias.
  - all matmuls bf16 (host pre-casts inputs; measured rel err ~3e-3 vs fp32).
  - reductions inner-contiguous (stage W halve+reduce, stage H contiguous
    binary tree over h); no strided elementwise ops.
  - per-engine instruction streams are FIFO, so batch-stage PHASES are
    interleaved at emission time to keep the PE fed during reduce chains.
  - GPSIMD does nothing (it contends with DVE for the shared SBUF port).
"""

import numpy as np
import ml_dtypes
from contextlib import ExitStack

import concourse.bass as bass
import concourse.bacc as bacc
import concourse.tile as tile
import concourse.mybir as mybir
from concourse.bass_utils import run_bass_kernel_spmd

F32 = mybir.dt.float32
BF16 = mybir.dt.bfloat16
AF = mybir.ActivationFunctionType
ALU = mybir.AluOpType
AX = mybir.AxisListType
NPBF = ml_dtypes.bfloat16

B, C, H, W = 16, 256, 64, 64
HW = H * W
NCORES = 8
BPC = B // NCORES
KT = 2
CH = 512
NCH = HW // CH
GRP = CH // 64

_BUILD_CACHE = {}
LAST_RESULTS = None


class _Stage:
    """Emits one attention stage (one batch) in three phases."""

    def __init__(self, nc, pools, axis_w, srcs, stat, wk, fus, bias, dst_evict):
        self.nc, self.axis_w = nc, axis_w
        self.phv = pools[-1]
        self.srcs, self.stat, self.wk, self.fus = srcs, stat, wk, fus
        self.bias, self.dst_evict = bias, dst_evict
        (self.pbig, self.pgate, self.pchunk, self.pctx, self.pq, self.pv, self.pf) = pools[:7]

    def p1_alloc(self):
        self.E = self.pbig.tile([128, HW], BF16, tag="E")
        self.gate = self.pgate.tile([128, 2, HW], BF16, tag="gate")
        self.u = self.pbig.tile([128, 2, HW], BF16, tag="u")

    def p1_chunk(self, j):
        nc, srcs, stat, bias = self.nc, self.srcs, self.stat, self.bias
        E, gate, u = self.E, self.gate, self.u
        bv2 = bias.get("bv2")
        if True:
            sl = bass.ts(j, CH)
            ps_q = self.pq.tile([128, CH], F32, tag="q")
            ps_v = self.pv.tile([128, 2 * CH], F32, tag="vf")
            for kt in range(KT):
                st, sp = kt == 0, kt == KT - 1
                rhs = srcs[j][:, kt, :]
                nc.tensor.matmul(ps_q[:], stat[:, kt, 2, :], rhs, start=st, stop=sp)
                nc.tensor.matmul(ps_v[:, 0:CH], stat[:, kt, 0, :], rhs, start=st, stop=sp)
                nc.tensor.matmul(ps_v[:, CH:], stat[:, kt, 1, :], rhs, start=st, stop=sp)
            nc.scalar.activation(E[:, sl], ps_q[:], AF.Exp, bias=bias["zb"])
            if bv2 is None:
                nc.scalar.activation(
                    gate[:, :, sl], ps_v[:].rearrange("p (c n) -> p c n", c=2),
                    AF.Tanh, bias=bias["zb"], scale=0.5,
                )
            else:
                nc.scalar.activation(gate[:, 0, sl], ps_v[:, 0:CH], AF.Tanh, bias=bv2[0], scale=0.5)
                nc.scalar.activation(gate[:, 1, sl], ps_v[:, CH:], AF.Tanh, bias=bv2[1], scale=0.5)
            eb = E[:, sl].unsqueeze(1).broadcast_to([128, 2, CH])
            nc.vector.tensor_tensor(u[:, :, sl], srcs[j][:, :, :], eb, op=ALU.mult)
            if not self.axis_w:
                nc.vector.tensor_scalar_add(gate[:, :, sl], gate[:, :, sl], 1.0)

    def _reduce64(self, flat, tag):
        nc, pctx = self.nc, self.pctx
        if self.axis_w:
            v3 = flat.rearrange("p (a r) -> p a r", r=64)
            hv = self.phv.tile([128, 64, 32], BF16, tag="hv")
            nc.vector.tensor_tensor(hv[:], v3[:, :, 0:32], v3[:, :, 32:64], op=ALU.add)
            h2 = self.phv.tile([128, 64, 16], BF16, tag="hv2")
            nc.vector.tensor_tensor(h2[:], hv[:, :, 0:16], hv[:, :, 16:32], op=ALU.add)
            nc.vector.tensor_tensor(h2[:, :, 0:8], h2[:, :, 0:8], h2[:, :, 8:16], op=ALU.add)
            out = pctx.tile([128, 64], F32, tag=f"red_{tag}")
            nc.vector.tensor_reduce(out[:], h2[:, :, 0:8], axis=AX.X, op=ALU.add)
        else:
            t = self.phv.tile([128, 2048], BF16, tag="tree2")
            nc.vector.tensor_tensor(t[:], flat[:, 0:2048], flat[:, 2048:4096], op=ALU.add)
            n = 1024
            while n >= 128:
                nc.vector.tensor_tensor(t[:, 0:n], t[:, 0:n], t[:, n : 2 * n], op=ALU.add)
                n //= 2
            out = pctx.tile([128, 64], F32, tag=f"red_{tag}")
            nc.vector.tensor_tensor(out[:], t[:, 0:64], t[:, 64:128], op=ALU.add)
        return out

    def p2(self):
        nc, pctx, bias = self.nc, self.pctx, self.bias
        S = self._reduce64(self.E[:], "S")
        R = pctx.tile([128, 64], F32, tag="R")
        nc.vector.reciprocal(R[:], S[:])
        xen = []
        if self.axis_w:
            xes = [self._reduce64(self.u[:, ct, :], f"xe{ct}") for ct in range(2)]
        else:
            # merged tree over both c-tiles: [128, 2, n] contiguous views
            t = self.phv.tile([128, 2, 2048], BF16, tag="tree2")
            u = self.u
            nc.vector.tensor_tensor(t[:], u[:, :, 0:2048], u[:, :, 2048:4096], op=ALU.add)
            n = 1024
            while n >= 128:
                nc.vector.tensor_tensor(t[:, :, 0:n], t[:, :, 0:n], t[:, :, n:2*n], op=ALU.add)
                n //= 2
            xep = self.pctx.tile([128, 2, 64], F32, tag="xep")
            nc.vector.tensor_tensor(xep[:], t[:, :, 0:64], t[:, :, 64:128], op=ALU.add)
            xes = [xep[:, ct, :] for ct in range(2)]
        for ct in range(2):
            xn = pctx.tile([128, 64], BF16, tag=f"xn{ct}")
            nc.vector.tensor_tensor(xn[:], xes[ct], R[:], op=ALU.mult)
            xen.append(xn)
        self.ctxs = []
        bk2 = bias.get("bk2")
        for mt in range(2):
            ps_c = self.pq.tile([128, 64], F32, tag="q")
            for ct in range(2):
                nc.tensor.matmul(ps_c[:], self.wk[:, ct, mt, :], xen[ct][:], start=ct == 0, stop=ct == 1)
            cn = pctx.tile([128, 64], BF16, tag=f"cn{mt}")
            if bk2 is None:
                nc.vector.tensor_scalar_mul(cn[:], ps_c[:], 0.5)
            else:
                nc.vector.tensor_scalar(cn[:], ps_c[:], 0.5, bk2[mt], op0=ALU.mult, op1=ALU.add)
            self.ctxs.append(cn)

    def p3_chunk(self, j):
        nc, srcs, fus = self.nc, self.srcs, self.fus
        if True:
            sl = bass.ts(j, CH)
            g2s = []
            for ct in range(2):
                if self.axis_w:
                    cb = self.ctxs[ct][:, bass.ts(j, GRP)].unsqueeze(2).broadcast_to([128, GRP, 64])
                else:
                    cb = self.ctxs[ct][:].unsqueeze(1).broadcast_to([128, GRP, 64])
                g2 = self.pchunk.tile([128, GRP, 64], BF16, tag=f"g2_{ct}")
                gv = self.gate[:, ct, sl].rearrange("p (a r) -> p a r", r=64)
                if self.axis_w:
                    nc.vector.scalar_tensor_tensor(g2[:], gv, 1.0, cb, op0=ALU.add, op1=ALU.mult)
                else:
                    nc.vector.tensor_tensor(g2[:], gv, cb, op=ALU.mult)
                g2s.append(g2)
            ps_f = self.pf.tile([128, 2 * CH], F32, tag="vf")
            for mt in range(2):
                half = ps_f[:, bass.ts(mt, CH)]
                nc.tensor.matmul(half, fus[:, 0, mt, :], srcs[j][:, 0, :], start=True, stop=False)
                nc.tensor.matmul(half, fus[:, 1, mt, :], srcs[j][:, 1, :], start=False, stop=False)
                nc.tensor.matmul(half, fus[:, 0, mt, :], g2s[0][:].rearrange("p a r -> p (a r)"), start=False, stop=False)
                nc.tensor.matmul(half, fus[:, 1, mt, :], g2s[1][:].rearrange("p a r -> p (a r)"), start=False, stop=True)
            self.dst_evict(j, ps_f)


def _build(flags):
    bvW0, bkW0, bvH0, bkH0, bfW0, bfH0 = flags
    nc = bacc.Bacc(trn_type="TRN2", target_bir_lowering=False, debug=False)

    x_d = nc.dram_tensor("x", [BPC, C, HW], BF16, kind="ExternalInput").ap()
    statW_d = nc.dram_tensor("statW", [128, KT, 3, 128], BF16, kind="ExternalInput").ap()
    statH_d = nc.dram_tensor("statH", [128, KT, 3, 128], BF16, kind="ExternalInput").ap()
    wkW_d = nc.dram_tensor("wkW", [128, KT, 2, 128], BF16, kind="ExternalInput").ap()
    wkH_d = nc.dram_tensor("wkH", [128, KT, 2, 128], BF16, kind="ExternalInput").ap()
    fusW_d = nc.dram_tensor("fusW", [128, KT, 2, 128], BF16, kind="ExternalInput").ap()
    fusH_d = nc.dram_tensor("fusH", [128, KT, 2, 128], BF16, kind="ExternalInput").ap()
    bias_d = nc.dram_tensor("biases", [6, 2, 128], F32, kind="ExternalInput").ap()
    y_d = nc.dram_tensor("y", [BPC, C, HW], BF16, kind="ExternalOutput").ap()

    with tile.TileContext(nc) as tc, ExitStack() as ctx:
        wp = ctx.enter_context(tc.tile_pool(name="weights", bufs=1))
        xbp = ctx.enter_context(tc.tile_pool(name="xbf", bufs=18))
        xwp = ctx.enter_context(tc.tile_pool(name="xw", bufs=16))
        pbig = ctx.enter_context(tc.tile_pool(name="big", bufs=2))
        pgate = ctx.enter_context(tc.tile_pool(name="gate", bufs=2))
        pchunk = ctx.enter_context(tc.tile_pool(name="chunk", bufs=3))
        pctx = ctx.enter_context(tc.tile_pool(name="ctx", bufs=3))
        phv = ctx.enter_context(tc.tile_pool(name="hv", bufs=2))
        yp = ctx.enter_context(tc.tile_pool(name="yev", bufs=3))
        pq = ctx.enter_context(tc.tile_pool(name="psq", bufs=2, space="PSUM"))
        pvf = ctx.enter_context(tc.tile_pool(name="psvf", bufs=3, space="PSUM"))
        pools = (pbig, pgate, pchunk, pctx, pq, pvf, pvf, phv)

        def wload(name, dram, shape, dt):
            t = wp.tile(shape, dt, tag=name)
            nc.scalar.dma_start(t[:], dram[:])
            return t

        statW = wload("statW", statW_d, [128, KT, 3, 128], BF16)
        statH = wload("statH", statH_d, [128, KT, 3, 128], BF16)
        wkW = wload("wkW", wkW_d, [128, KT, 2, 128], BF16)
        wkH = wload("wkH", wkH_d, [128, KT, 2, 128], BF16)
        fusW = wload("fusW", fusW_d, [128, KT, 2, 128], BF16)
        fusH = wload("fusH", fusH_d, [128, KT, 2, 128], BF16)

        bias_sb = wp.tile([128, 6, 2], F32, tag="biases")
        nc.scalar.dma_start(bias_sb[:], bias_d[:].transpose([2, 0, 1]))
        zb = wp.tile([128, 1], F32, tag="zb")
        nc.vector.memset(zb[:], 0.0)

        def bap(i, ct):
            return bias_sb[:, i, ct].unsqueeze(1)

        biasW = {
            "bv2": None if bvW0 else [bap(0, ct) for ct in range(2)],
            "bk2": None if bkW0 else [bap(1, ct) for ct in range(2)],
            "zb": zb[:],
        }
        biasH = {
            "bv2": None if bvH0 else [bap(2, ct) for ct in range(2)],
            "bk2": None if bkH0 else [bap(3, ct) for ct in range(2)],
            "zb": zb[:],
        }

        def load_x(b):
            xcs = []
            for j in range(NCH):
                xc = xbp.tile([128, KT, CH], BF16, tag="xc")
                for kt in range(KT):
                    nc.sync.dma_start(xc[:, kt, :], x_d[b, bass.ts(kt, 128), bass.ts(j, CH)])
                xcs.append(xc[:])
            return xcs

        def make_W(b, xcs):
            xw_tiles = [None] * NCH

            def evW(j, ps_f):
                xw = xwp.tile([128, KT, CH], BF16, tag="xw")
                xw_tiles[j] = xw[:]
                if bfW0:
                    nc.scalar.activation(xw[:], ps_f[:].rearrange("p (c n) -> p c n", c=2), AF.Copy)
                else:
                    for ct in range(2):
                        nc.scalar.activation(
                            xw[:, ct, :], ps_f[:, bass.ts(ct, CH)],
                            AF.Identity, bias=bap(4, ct),
                        )

            st = _Stage(nc, pools, True, xcs, statW, wkW, fusW, biasW, evW)
            st.xw_tiles = xw_tiles
            return st

        def make_H(b, xw_tiles):
            def evH(j, ps_f):
                y_t = yp.tile([128, 2, CH], BF16, tag="y")
                if bfH0:
                    nc.scalar.activation(y_t[:], ps_f[:].rearrange("p (c n) -> p c n", c=2), AF.Copy)
                else:
                    for ct in range(2):
                        nc.scalar.activation(
                            y_t[:, ct, :], ps_f[:, bass.ts(ct, CH)],
                            AF.Identity, bias=bap(5, ct),
                        )
                nc.sync.dma_start(
                    y_d[b].rearrange("(c p) n -> p c n", p=128)[:, :, bass.ts(j, CH)],
                    y_t[:],
                )

            return _Stage(nc, pools, False, xw_tiles, statH, wkH, fusH, biasH, evH)

        # interleaved phase schedule: chunk-level alternation keeps every
        # engine's FIFO stream fed during the other phase's stalls
        def run_p1(st):
            st.p1_alloc()
            for j in range(NCH):
                st.p1_chunk(j)

        def run_p3(st):
            for j in range(NCH):
                st.p3_chunk(j)

        x0 = load_x(0)
        x1 = load_x(1)
        w0 = make_W(0, x0)
        w1 = make_W(1, x1)
        run_p1(w0)
        run_p1(w1)
        w0.p2()
        run_p3(w0)
        h0 = make_H(0, w0.xw_tiles)
        w1.p2()
        run_p1(h0)
        run_p3(w1)
        h1 = make_H(1, w1.xw_tiles)
        h0.p2()
        run_p1(h1)
        run_p3(h0)
        h1.p2()
        run_p3(h1)

    nc.compile()
    return nc


def _prep(qkv_w, fus_w):
    wq = qkv_w[0]
    wk = qkv_w[1 : 1 + C]
    wv = qkv_w[1 + C :]
    stat = np.empty((128, KT, 3, 128), np.float32)
    wkt = np.empty((128, KT, 2, 128), np.float32)
    fus = np.empty((128, KT, 2, 128), np.float32)
    for kt in range(KT):
        cs = slice(kt * 128, (kt + 1) * 128)
        stat[:, kt, 0, :] = wv[0:128, cs].T
        stat[:, kt, 1, :] = wv[128:256, cs].T
        stat[:, kt, 2, :] = np.repeat(wq[cs][:, None], 128, axis=1)
        wkt[:, kt, 0, :] = wk[0:128, cs].T
        wkt[:, kt, 1, :] = wk[128:256, cs].T
        fus[:, kt, 0, :] = fus_w[0:128, cs].T
        fus[:, kt, 1, :] = fus_w[128:256, cs].T
    tobf = lambda a: np.ascontiguousarray(a.astype(NPBF))
    return tobf(stat), tobf(wkt), tobf(fus)


def kernel(x, qkvW_w, qkvW_b, qkvH_w, qkvH_b, fusW_w, fusW_b, fusH_w, fusH_b):
    global LAST_RESULTS
    x = np.asarray(x, np.float32)
    qkvW_w = np.asarray(qkvW_w, np.float32)
    qkvW_b = np.asarray(qkvW_b, np.float32)
    qkvH_w = np.asarray(qkvH_w, np.float32)
    qkvH_b = np.asarray(qkvH_b, np.float32)
    fusW_w = np.asarray(fusW_w, np.float32)
    fusW_b = np.asarray(fusW_b, np.float32)
    fusH_w = np.asarray(fusH_w, np.float32)
    fusH_b = np.asarray(fusH_b, np.float32)

    statW, wkW, fusW = _prep(qkvW_w, fusW_w)
    statH, wkH, fusH = _prep(qkvH_w, fusH_w)

    bkW = qkvW_b[1 : 1 + C]
    bvW = qkvW_b[1 + C :]
    bkH = qkvH_b[1 : 1 + C]
    bvH = qkvH_b[1 + C :]
    biases = np.stack(
        [0.5 * bvW.reshape(2, 128),
         0.5 * bkW.reshape(2, 128),
         0.5 * bvH.reshape(2, 128),
         0.5 * bkH.reshape(2, 128),
         fusW_b.reshape(2, 128),
         fusH_b.reshape(2, 128)]
    ).astype(np.float32)

    flags = (
        not bvW.any(), not bkW.any(), not bvH.any(), not bkH.any(),
        not fusW_b.any(), not fusH_b.any(),
    )
    if flags not in _BUILD_CACHE:
        _BUILD_CACHE[flags] = _build(flags)
    nc = _BUILD_CACHE[flags]

    xbf = np.ascontiguousarray(x.reshape(B, C, HW).astype(NPBF))
    in_maps = []
    for core in range(NCORES):
        in_maps.append({
            "x": xbf[core * BPC : (core + 1) * BPC],
            "statW": statW, "statH": statH,
            "wkW": wkW, "wkH": wkH, "fusW": fusW, "fusH": fusH,
            "biases": biases,
        })

    res = run_bass_kernel_spmd(nc, in_maps, list(range(NCORES)))
    LAST_RESULTS = res
    y = np.concatenate([r["y"] for r in res.results], axis=0)
    return y.astype(np.float32).reshape(B, C, H, W)

